# revision 55
# baseline (speedup 1.0000x reference)
"""Dilated multi-head attention (nn_DilatedMHA) on 8 trn2 NeuronCores.

Math (reference restructured):
  qkv = x @ Wqkv.T                      [b, n, 3, h, d]   b=2, n=8192, h=12, d=64
  Position i attends within its mod-2048 class {p, p+2048, p+4096, p+6144}
  (p = i % 2048).  Per group p and head: r=1 full 4x4 softmax; r=2 (p even)
  2x2 among same-parity slots; r=4 (p%4==0) adds v.  out is normalized by
  its sum over the sequence per (b, h*d) channel, then projected by Wout.

Sharding: core c <- batch c//4, groups p in [(c%4)*512, (c%4)*512+512).

Key precision structure: the normalization denominator s nearly cancels
(min |s| ~ 0.018 vs ~0.5 summands), so anything that feeds s is amplified
~1000x into the output.  Instead of summing the (possibly noisy) attention
output, s is computed on an exact side channel:
    s[h,d] = sum_p sum_jp C[p,h,jp] * (x[p,jp,:] @ Wv[:, (h,d)])
           = gT.T @ Wv-diag-blocks,   gT = sum_chunks x_chunk.T @ C_chunk
where C = sum_j Wt are the attention-weight column sums (fp32).  C needs
~2^-15-accurate scores; Q/K are projected as xh@wh in fp16 (1 cyc/col)
plus BOTH hi/lo cross terms in one fp8e5m2 DoubleRow matmul (0.5 cyc/col,
two K-slots per PE cell): slot0 = (xh/1024)@(wl*1024), slot1 =
(xl*32)@(wh/32) - per-operand scales cancel within each slot and keep the
tiny residuals inside e5m2's normal range.  Measured on hardware this adds
only ~1e-3 of output error (the numpy e5m2 emulation predicted 1.3e-2 -
the real PE is kinder than the emulation).

With s exact, every other path only needs ~12-bit relative accuracy and runs
at the PE's full 1 cycle/column fp16 rate: the V projection is a single fp16
matmul, AV/oacc/oT/out-projection are fp16 (2x DVE throughput, 1 cyc/col
transposes), and Wout is fp16 with 1/s folded in per channel.

Structure: pass 0 = Q,K 3-term projections + per-pair scores (Pool muls /
DVE reduces chase each chunk's PSUM evacuation) + softmax/Wt/C + g; the
s side channel + AllGather (1.875x cheaper than AllReduce) then overlap
pass 1 = V + AV + transposes + out-projection.  Block 3's g-matmuls are
deferred into pass 1 so their ~10us wait on C3 does not overflow the PE's
4-deep wait queue at the pass boundary.  PSUM start/stop flags ride on
full-tile zero-init matmuls because the tile scheduler freely reorders
per-slice accumulating matmuls (a slice-level `start` can execute late and
wipe earlier contributions).

Measured: rel err 1.9e-3 on hardware (budget 2e-2); cost-model device
time 219610 ns vs 415908 ns baseline (1.89x).  PE busy 179us of 220us
(81%): QK 92us (fp16 + fp8-DR), V 31us, out-proj 31us, g 15us,
transposes 5us.  Remaining idle: ~12us DMA head fill, ~15us collective
window (pass 1 drains before ws is ready), drain.  Known rejected
variants: DMA-XBAR transposes for oT, fp32 s-dot, AllReduce, sliced
weight-DMA priority, batched-by-jp scores in the last block - each
re-shuffles the tile scheduler into a worse global order (+2..15us).
A 3-term fp16 QK fallback (rel err 9.0e-4, 308505 ns) is kernel_308.py.
"""

import sys

if "/opt/trn_rl_repo" not in sys.path:
    sys.path.insert(0, "/opt/trn_rl_repo")

import numpy as np

EMBED = 768
HEADS = 12
HD = 64
B = 2
N = 8192
NCORES = 8
GPC = 512           # groups per core
NBLK = 4            # blocks of 128 groups per core
NCHUNK = 16         # row chunks of 128 per core (blk, j)
KO = 6              # embed // 128
QKW = 2 * EMBED     # Q|K output columns

_COMPILED = {}


def _build_program():
    import concourse.mybir as mybir
    import concourse.tile as tile
    from concourse import bacc

    F32 = mybir.dt.float32
    F16 = mybir.dt.float16
    F8 = mybir.dt.float8e5
    DR = mybir.MatmulPerfMode.DoubleRow
    AX = mybir.AxisListType
    OP = mybir.AluOpType
    ACTF = mybir.ActivationFunctionType

    nc = bacc.Bacc("TRN2", target_bir_lowering=False, debug=False, num_devices=NCORES)

    # --- DRAM I/O ---------------------------------------------------------
    xch_d = nc.dram_tensor("xch", [NCHUNK, 128, KO, 128], F16, kind="ExternalInput")
    x8_d = nc.dram_tensor("x8", [NCHUNK, 128, KO, 2, 128], F8, kind="ExternalInput")
    xrh_d = nc.dram_tensor("xrh", [NCHUNK, 128, KO, 128], F16, kind="ExternalInput")
    xrl_d = nc.dram_tensor("xrl", [NCHUNK, 128, KO, 128], F16, kind="ExternalInput")
    wqh_d = nc.dram_tensor("wqh", [128, KO, QKW], F16, kind="ExternalInput")
    wq8_d = nc.dram_tensor("wq8", [128, KO, 2, QKW], F8, kind="ExternalInput")
    wvh_d = nc.dram_tensor("wvh", [128, KO, EMBED], F16, kind="ExternalInput")
    wvl_d = nc.dram_tensor("wvl", [128, KO, EMBED], F16, kind="ExternalInput")
    wo_d = nc.dram_tensor("wo16", [128, KO, EMBED], F16, kind="ExternalInput")
    m2_d = nc.dram_tensor("m2", [128, 1], F32, kind="ExternalInput")
    m4_d = nc.dram_tensor("m4", [128, 1], F32, kind="ExternalInput")
    id_d = nc.dram_tensor("ident16", [128, 128], F16, kind="ExternalInput")
    id32_d = nc.dram_tensor("ident32", [128, 128], F32, kind="ExternalInput")
    y_d = nc.dram_tensor("y", [4 * GPC, EMBED], F32, kind="ExternalOutput")

    with tile.TileContext(nc) as tc:
        with (
            tc.tile_pool(name="const", bufs=1) as constp,
            tc.tile_pool(name="oT", bufs=4) as oTp,
            tc.tile_pool(name="wt16", bufs=4) as wt16p,
            tc.tile_pool(name="xch", bufs=NCHUNK) as xchp,
            tc.tile_pool(name="mm", bufs=3, space="PSUM") as mmp,
            tc.tile_pool(name="gp", bufs=1, space="PSUM") as gpp,
            tc.tile_pool(name="dram", bufs=2, space="DRAM") as dramp,
            tc.tile_pool(name="xr", bufs=6) as xrp,
            tc.tile_pool(name="att", bufs=1) as attp,
        ):
            # --- long-lived SBUF -----------------------------------------
            wqh_sb = constp.tile([128, KO, QKW], F16)
            wq8_sb = constp.tile([128, KO, 2, QKW], F8)
            wvh_sb = constp.tile([128, KO, EMBED], F16)
            wvl_sb = constp.tile([128, KO, EMBED], F16)
            wo_sb = constp.tile([128, KO, EMBED], F16)
            m2_sb = constp.tile([128, 1], F32)
            nc.sync.dma_start(m2_sb[:], m2_d[:])
            m4_sb = constp.tile([128, 1], F32)
            nc.sync.dma_start(m4_sb[:], m4_d[:])
            id_sb = constp.tile([128, 128], F16)
            nc.sync.dma_start(id_sb[:], id_d[:])
            id32_sb = constp.tile([128, 128], F32)
            nc.sync.dma_start(id32_sb[:], id32_d[:])
            zero384 = constp.tile([128, 384], F16)
            nc.vector.memset(zero384[:], 0.0)

            # g accumulator: g[h, e] = sum_p C[p, h] x[p, e], two 384-wide
            # PSUM halves, alive all of pass 0.  A full-tile zero-init matmul
            # carries the single `start` so every accumulating matmul has a
            # data dependency on it (the scheduler is otherwise free to
            # reorder, which would run the start late and wipe contributions).
            gpsA = gpp.tile([12, 384], F32, tag="ga")
            gpsB = gpp.tile([12, 384], F32, tag="gb")
            nc.tensor.matmul(
                gpsA[:], lhsT=id_sb[:, 0:12], rhs=zero384[:],
                start=True, stop=False,
            )
            nc.tensor.matmul(
                gpsB[:], lhsT=id_sb[:, 0:12], rhs=zero384[:],
                start=True, stop=False,
            )

            oT_blocks = []
            wt16_blocks = []
            xch_tiles = {}
            deferred_g = []

            def emit_g(blk, xhl, Ch, Cl):
                for j in range(4):
                    _, _, xrh_sb, xrl_sb = xhl[j]
                    xrh2 = xrh_sb[:].rearrange("p a b -> p (a b)")
                    xrl2 = xrl_sb[:].rearrange("p a b -> p (a b)")
                    last = blk == NBLK - 1 and j == 3
                    chj = Ch[:, :, j:j + 1].rearrange("p h a -> p (h a)")
                    clj = Cl[:, :, j:j + 1].rearrange("p h a -> p (h a)")
                    for half, gt in ((0, gpsA), (1, gpsB)):
                        sl = slice(half * 384, (half + 1) * 384)
                        nc.tensor.matmul(
                            gt[:], lhsT=chj, rhs=xrh2[:, sl],
                            start=False, stop=False,
                        )
                        nc.tensor.matmul(
                            gt[:], lhsT=clj, rhs=xrh2[:, sl],
                            start=False, stop=False,
                        )
                        nc.tensor.matmul(
                            gt[:], lhsT=chj, rhs=xrl2[:, sl],
                            start=False, stop=last,
                        )

            def load_xch(chunk):
                t = xchp.tile([128, KO, 128], F16, tag="xch")
                nc.sync.dma_start(t[:], xch_d[chunk])
                xch_tiles[chunk] = t
                return t

            # ============ Pass 0: Q,K + scores + C + g + s ===============
            with (
                tc.tile_pool(name="xcl", bufs=3) as xclp,
                tc.tile_pool(name="qkv", bufs=2) as qkvp,
                tc.tile_pool(name="prs", bufs=1) as prsp,
            ):
                def load_xr(chunk):
                    xrh_sb = xrp.tile([128, KO, 128], F16, tag="xrh")
                    nc.sync.dma_start(xrh_sb[:], xrh_d[chunk])
                    xrl_sb = xrp.tile([128, KO, 128], F16, tag="xrl")
                    nc.sync.dma_start(xrl_sb[:], xrl_d[chunk])
                    return xrh_sb, xrl_sb

                def load_qk_x(chunk):
                    xh_sb = load_xch(chunk)
                    x8_sb = xclp.tile([128, KO, 2, 128], F8, tag="x8")
                    nc.sync.dma_start(x8_sb[:], x8_d[chunk])
                    return xh_sb, x8_sb

                def load_chunk(chunk):
                    return load_qk_x(chunk) + load_xr(chunk)

                # DMA priority: chunk-0 QK x, QK weights, chunk 1, xr later.
                h0 = load_qk_x(0)
                for ko in range(KO):
                    nc.sync.dma_start(wqh_sb[:, ko, :], wqh_d[:, ko, :])
                    nc.sync.dma_start(wq8_sb[:, ko, :, :], wq8_d[:, ko, :, :])
                h1 = load_qk_x(1)
                pre = {0: h0 + load_xr(0), 1: h1 + load_xr(1)}

                for blk in range(NBLK):
                    if blk == 2:
                        # pass-1 weights, off the critical path at both ends
                        nc.sync.dma_start(wvh_sb[:], wvh_d[:])
                        nc.sync.dma_start(wvl_sb[:], wvl_d[:])
                        nc.sync.dma_start(wo_sb[:], wo_d[:])
                    Qb = qkvp.tile([128, 4, EMBED], F32, tag="qb")
                    Kb = qkvp.tile([128, 4, EMBED], F32, tag="kb")
                    Q4 = Qb[:].rearrange("p j (h d) -> p j h d", d=HD)
                    K4 = Kb[:].rearrange("p j (h d) -> p j h d", d=HD)
                    S = attp.tile([128, 4, HEADS, 4], F32, tag="S")
                    xhl = []
                    npair = 0
                    for j in range(4):
                        chunk = blk * 4 + j
                        if chunk in pre:
                            tiles = pre.pop(chunk)
                        else:
                            tiles = load_chunk(chunk)
                        if chunk + 2 < NCHUNK and (chunk + 2) not in pre:
                            pre[chunk + 2] = load_chunk(chunk + 2)
                        xh_sb, x8_sb, xrh_sb, xrl_sb = tiles
                        xhl.append(tiles)
                        # Q|K: fp16 main term + fp8e5 DoubleRow cross terms
                        # (slot0 = (xh/1024)@(wl*1024), slot1 = (xl*32)@(wh/32))
                        for n in range(4):
                            ps = mmp.tile([128, 384], F32, tag="mm")
                            sl = slice(n * 384, (n + 1) * 384)
                            for ko in range(KO):
                                nc.tensor.matmul(
                                    ps[:], lhsT=xh_sb[:, ko, :],
                                    rhs=wqh_sb[:, ko, sl],
                                    start=(ko == 0), stop=False,
                                )
                                nc.tensor.matmul(
                                    ps[:], lhsT=x8_sb[:, ko, :, :],
                                    rhs=wq8_sb[:, ko, :, sl],
                                    perf_mode=DR,
                                    start=False, stop=(ko == KO - 1),
                                )
                            dest = Qb if n < 2 else Kb
                            nc.scalar.copy(
                                dest[:, j, (n % 2) * 384:(n % 2 + 1) * 384], ps[:]
                            )

                        # per-pair scores chase the chunk evacuations
                        pairs = [(b_, j) for b_ in range(j + 1)]
                        pairs += [(j, c_) for c_ in range(j)]
                        for b_, c_ in pairs:
                            pr = prsp.tile(
                                [128, HEADS, HD], F32, tag=f"prs{npair % 2}"
                            )
                            nc.gpsimd.tensor_mul(pr[:], Q4[:, b_], K4[:, c_])
                            nc.vector.reduce_sum(
                                S[:, b_, :, c_], pr[:], axis=AX.X
                            )
                            npair += 1

                    E = attp.tile([128, 4, HEADS, 4], F32, tag="E")
                    nc.scalar.activation(E[:], S[:], ACTF.Exp, scale=0.125)

                    # --- attention weights Wt (fp32) ---------------------
                    Z1 = attp.tile([128, 4, HEADS], F32, tag="Z1")
                    nc.vector.reduce_sum(Z1[:], E[:], axis=AX.X)
                    R1 = attp.tile([128, 4, HEADS], F32, tag="R1")
                    nc.vector.reciprocal(R1[:], Z1[:])
                    Z2 = attp.tile([128, 4, HEADS, 2], F32, tag="Z2")
                    nc.vector.tensor_add(Z2[:], E[:, :, :, 0:2], E[:, :, :, 2:4])
                    R2 = attp.tile([128, 4, HEADS, 2], F32, tag="R2")
                    nc.vector.reciprocal(R2[:], Z2[:])

                    W1 = attp.tile([128, 4, HEADS, 4], F32, tag="W1")
                    nc.vector.tensor_mul(
                        W1[:], E[:], R1[:, :, :, None].to_broadcast((128, 4, HEADS, 4))
                    )
                    W2 = attp.tile([128, 4, HEADS, 4], F32, tag="W2")
                    nc.vector.memset(W2[:], 0.0)
                    for par in (0, 1):
                        nc.vector.tensor_mul(
                            W2[:, par::2, :, par::2],
                            E[:, par::2, :, par::2],
                            R2[:, par::2, :, par:par + 1].to_broadcast(
                                (128, 2, HEADS, 2)
                            ),
                        )
                    Wt = attp.tile([128, 4, HEADS, 4], F32, tag="Wt")
                    nc.vector.scalar_tensor_tensor(
                        Wt[:], W2[:], m2_sb[:, 0:1], W1[:], OP.mult, OP.add
                    )
                    for j in range(4):
                        nc.vector.tensor_scalar_add(
                            Wt[:, j, :, j:j + 1], Wt[:, j, :, j:j + 1], m4_sb[:, 0:1]
                        )
                    Wt16 = wt16p.tile([128, 4, HEADS, 4], F16, tag="wt16")
                    nc.scalar.copy(Wt16[:], Wt[:])
                    wt16_blocks.append(Wt16)

                    # --- C = column sums of Wt (exact, fp32) -------------
                    C = attp.tile([128, HEADS, 4], F32, tag="C")
                    nc.vector.reduce_sum(
                        C[:], Wt[:].rearrange("p j h k -> p h k j"), axis=AX.X
                    )
                    Ch = attp.tile([128, HEADS, 4], F16, tag="Ch")
                    nc.scalar.copy(Ch[:], C[:])
                    Cl = attp.tile([128, HEADS, 4], F16, tag="Cl")
                    nc.vector.tensor_sub(Cl[:], C[:], Ch[:])

                    # --- g[h, e] += C_chunk[p, h]^T @ x_chunk[p, e] -------
                    # 3-term fp16; xr tiles are row-major [p, (ko, e)].
                    # Block 3's g-matmuls wait ~10us on its C; deferring them
                    # into pass 1 keeps the PE wait queue from blocking the
                    # pass boundary.
                    if blk < NBLK - 1:
                        emit_g(blk, xhl, Ch, Cl)
                    else:
                        deferred_g.append((blk, xhl, Ch, Cl))

            # =============== exact s + AllGather (emitted mid pass 1) ====
            def emit_s_tail():
                for args in deferred_g:
                    emit_g(*args)
                g_row = constp.tile([12, 2, 384], F32)
                nc.scalar.copy(g_row[:, 0, :], gpsA[:])
                nc.scalar.copy(g_row[:, 1, :], gpsB[:])
                g2 = g_row[:].rearrange("h a b -> h (a b)")
                g_sb = constp.tile([128, KO, HEADS], F32)
                for ko in range(KO):
                    gt_ps = gpp.tile([128, HEADS], F32, tag="gt")
                    nc.tensor.transpose(
                        gt_ps[:], g2[:, ko * 128:(ko + 1) * 128],
                        id32_sb[0:12, 0:12],
                    )
                    nc.scalar.copy(g_sb[:, ko, :], gt_ps[:])
                gh = constp.tile([128, KO, HEADS], F16)
                nc.scalar.copy(gh[:], g_sb[:])
                gl = constp.tile([128, KO, HEADS], F16)
                nc.vector.tensor_sub(gl[:], g_sb[:], gh[:])

                # s matmuls accumulate into a gp-pool tile (tag reuse keeps
                # PSUM within 8 banks)
                stp = gpp.tile([128, KO, 2], F32, tag="gt")
                nc.tensor.matmul(
                    stp[:].rearrange("p a b -> p (a b)"), lhsT=id_sb[:],
                    rhs=zero384[:, 0:KO * 2], start=True, stop=False,
                )
                for t in range(KO):
                    sl = slice(t * 128, (t + 1) * 128)
                    hs = slice(2 * t, 2 * t + 2)
                    for ko in range(KO):
                        nc.tensor.matmul(
                            stp[:, t, :], lhsT=wvh_sb[:, ko, sl],
                            rhs=gh[:, ko, hs], start=False, stop=False,
                        )
                        nc.tensor.matmul(
                            stp[:, t, :], lhsT=wvh_sb[:, ko, sl],
                            rhs=gl[:, ko, hs], start=False, stop=False,
                        )
                        nc.tensor.matmul(
                            stp[:, t, :], lhsT=wvl_sb[:, ko, sl],
                            rhs=gh[:, ko, hs], start=False,
                            stop=(t == KO - 1 and ko == KO - 1),
                        )
                s_chan = constp.tile([128, KO], F32)
                for t in range(KO):
                    nc.vector.tensor_copy(s_chan[0:64, t:t + 1], stp[0:64, t, 0:1])
                    nc.vector.tensor_copy(
                        s_chan[64:128, t:t + 1], stp[64:128, t, 1:2]
                    )

                # AllGather (1.875x cheaper than AllReduce in latency) of the
                # four quarter-core partial sums, then add locally.
                cc_in = dramp.tile([128, KO], F32)
                cc_out = dramp.tile([4, 128, KO], F32)
                nc.gpsimd.dma_start(cc_in[:], s_chan[:])
                nc.gpsimd.collective_compute(
                    "AllGather",
                    OP.bypass,
                    replica_groups=[[0, 1, 2, 3], [4, 5, 6, 7]],
                    ins=[cc_in[:].opt()],
                    outs=[cc_out[:].opt()],
                )
                s_gath = constp.tile([128, 4, KO], F32)
                nc.gpsimd.dma_start(
                    s_gath[:], cc_out[:].rearrange("g p t -> p g t")
                )
                sa = constp.tile([128, KO], F32)
                nc.vector.tensor_add(sa[:], s_gath[:, 0, :], s_gath[:, 1, :])
                sb2 = constp.tile([128, KO], F32)
                nc.vector.tensor_add(sb2[:], s_gath[:, 2, :], s_gath[:, 3, :])
                s_tot = constp.tile([128, KO], F32)
                nc.vector.tensor_add(s_tot[:], sa[:], sb2[:])
                r_sb = constp.tile([128, KO], F32)
                nc.vector.reciprocal(r_sb[:], s_tot[:])
                return r_sb

            # ====== Pass 1: V + AV + transposes (collective hidden) ======
            with (
                tc.tile_pool(name="vq", bufs=2) as vqp,
                tc.tile_pool(name="avt", bufs=2) as avtp,
                tc.tile_pool(name="oacc", bufs=2) as oaccp,
                tc.tile_pool(name="ws", bufs=1) as wsp,
                tc.tile_pool(name="fin", bufs=2) as finp,
                tc.tile_pool(name="tp", bufs=2, space="PSUM") as tpp,
            ):
                r_sb = None
                for blk in range(NBLK):
                    if blk == 1:
                        r_sb = emit_s_tail()
                    V16 = vqp.tile([128, 4, EMBED], F16, tag="vb")
                    for j in range(4):
                        xh_sb = xch_tiles[blk * 4 + j]
                        for n in range(2):
                            ps = mmp.tile([128, 384], F32, tag="mm")
                            sl = slice(n * 384, (n + 1) * 384)
                            for ko in range(KO):
                                nc.tensor.matmul(
                                    ps[:], lhsT=xh_sb[:, ko, :],
                                    rhs=wvh_sb[:, ko, sl],
                                    start=(ko == 0), stop=(ko == KO - 1),
                                )
                            nc.scalar.copy(V16[:, j, n * 384:(n + 1) * 384], ps[:])

                    # --- AV (fp16): oacc[p, j] = sum_jp Wt16 * V16 -------
                    # muls on Pool (flat 0.833ns/elem, broadcast-immune),
                    # adds on DVE (fp16 packed 2x)
                    Wt16 = wt16_blocks[blk]
                    oacc = oaccp.tile([128, 4, EMBED], F16, tag="oacc")
                    o4 = oacc[:].rearrange("p j (h d) -> p j h d", d=HD)
                    for jp in range(4):
                        vb = (
                            V16[:, jp:jp + 1, :]
                            .rearrange("p a (h d) -> p a h d", d=HD)
                            .to_broadcast((128, 4, HEADS, HD))
                        )
                        wb = Wt16[:, :, :, jp:jp + 1].to_broadcast(
                            (128, 4, HEADS, HD)
                        )
                        if jp == 0:
                            nc.vector.scalar_tensor_tensor(
                                o4[:], vb, 1.0, wb, OP.mult, OP.mult
                            )
                        else:
                            t = avtp.tile(
                                [128, 4, HEADS, HD], F16, tag=f"avt{jp % 2}"
                            )
                            nc.gpsimd.tensor_mul(t[:], vb, wb)
                            nc.vector.tensor_add(o4[:], o4[:], t[:])

                    # --- transpose oacc -> oT[hd, rows] (fp16, 1 cyc) ----
                    oT = oTp.tile([128, KO, 4 * 128], F16, tag="oT")
                    for j in range(4):
                        for ko in range(KO):
                            pt = tpp.tile([128, 128], F16, tag="tp")
                            nc.tensor.transpose(
                                pt[:], oacc[:, j, ko * 128:(ko + 1) * 128], id_sb[:]
                            )
                            if (j * KO + ko) % 3 == 0:
                                nc.vector.tensor_copy(
                                    oT[:, ko, j * 128:(j + 1) * 128], pt[:]
                                )
                            else:
                                nc.scalar.copy(
                                    oT[:, ko, j * 128:(j + 1) * 128], pt[:]
                                )
                    oT_blocks.append(oT)

                # =============== out-projection ==========================
                ws_sb = wsp.tile([128, KO, EMBED], F16)
                for ko in range(KO):
                    nc.vector.tensor_scalar_mul(
                        ws_sb[:, ko, :], wo_sb[:, ko, :], r_sb[:, ko:ko + 1]
                    )

                for blk in range(NBLK):
                    oT = oT_blocks[blk]
                    for rc in range(4):
                        for half in range(2):
                            pf = mmp.tile([128, 384], F32, tag="mm")
                            for ko in range(KO):
                                nc.tensor.matmul(
                                    pf[:],
                                    lhsT=oT[:, ko, rc * 128:(rc + 1) * 128],
                                    rhs=ws_sb[:, ko, half * 384:(half + 1) * 384],
                                    start=(ko == 0),
                                    stop=(ko == KO - 1),
                                )
                            fin = finp.tile([128, 384], F32, tag="fin")
                            nc.scalar.copy(fin[:], pf[:])
                            rows = blk * 512 + rc * 128
                            nc.sync.dma_start(
                                y_d[rows:rows + 128, half * 384:(half + 1) * 384],
                                fin[:],
                            )

    nc.finalize()
    return nc


def _host_shard(x, Wqkv, Wout):
    """Build per-core input maps."""
    x = np.ascontiguousarray(np.asarray(x, dtype=np.float32))
    Wqkv = np.asarray(Wqkv, dtype=np.float32)
    Wout = np.asarray(Wout, dtype=np.float32)

    wq = np.ascontiguousarray(
        Wqkv.T.reshape(KO, 128, 3 * EMBED).transpose(1, 0, 2)
    )
    import concourse.mybir as _mybir
    F8NP = _mybir.dt.np(_mybir.dt.float8e5)
    wqk = wq[:, :, :QKW]
    wqh = np.ascontiguousarray(wqk.astype(np.float16))
    wql32 = wqk - wqh.astype(np.float32)
    # wq8[:, ko, 0, :] = wl*1024 (pairs with xh/1024); [:, ko, 1, :] = wh/32
    wq8 = np.empty((128, KO, 2, QKW), dtype=F8NP)
    wq8[:, :, 0, :] = (wql32 * 1024.0).astype(F8NP)
    wq8[:, :, 1, :] = (wqh.astype(np.float32) / 32.0).astype(F8NP)
    wv = wq[:, :, QKW:]
    wvh = np.ascontiguousarray(wv.astype(np.float16))
    wvl = np.ascontiguousarray((wv - wvh.astype(np.float32)).astype(np.float16))
    wo16 = np.ascontiguousarray(
        Wout.T.reshape(KO, 128, EMBED).transpose(1, 0, 2).astype(np.float16)
    )
    m2 = (np.arange(128) % 2 == 0).astype(np.float32).reshape(128, 1)
    m4 = (np.arange(128) % 4 == 0).astype(np.float32).reshape(128, 1)
    ident16 = np.eye(128, dtype=np.float16)
    ident32 = np.eye(128, dtype=np.float32)

    in_maps = []
    for c in range(NCORES):
        bc, q = divmod(c, 4)
        xb = x[bc].reshape(4, 4, 4, 128, EMBED)  # [j, q, blk, g, e]
        mine = xb[:, q]                          # [j, blk, g, e]
        t = np.ascontiguousarray(mine.transpose(1, 0, 2, 3)).reshape(
            NCHUNK, 128, EMBED
        )
        xc = np.ascontiguousarray(
            t.reshape(NCHUNK, 128, KO, 128).transpose(0, 3, 2, 1)
        )
        xch = xc.astype(np.float16)
        xcl32 = xc - xch.astype(np.float32)
        x8 = np.empty((NCHUNK, 128, KO, 2, 128), dtype=F8NP)
        x8[:, :, :, 0, :] = (xch.astype(np.float32) / 1024.0).astype(F8NP)
        x8[:, :, :, 1, :] = (xcl32 * 32.0).astype(F8NP)
        xr = np.ascontiguousarray(t.reshape(NCHUNK, 128, KO, 128))
        xrh = xr.astype(np.float16)
        xrl = (xr - xrh.astype(np.float32)).astype(np.float16)
        in_maps.append(
            {
                "xch": xch, "x8": x8, "xrh": xrh, "xrl": xrl,
                "wqh": wqh, "wq8": wq8, "wvh": wvh, "wvl": wvl,
                "wo16": wo16, "m2": m2, "m4": m4, "ident16": ident16,
                "ident32": ident32,
            }
        )
    return in_maps


def _host_assemble(results):
    y = np.empty((B, N, EMBED), dtype=np.float32)
    for c in range(NCORES):
        bc, q = divmod(c, 4)
        yc = np.asarray(results[c]["y"])  # [2048, 768], rows (blk, j, g)
        part = yc.reshape(4, 4, 128, EMBED).transpose(1, 0, 2, 3)  # [j, blk, g, e]
        y[bc].reshape(4, 4, 4, 128, EMBED)[:, q] = part
    return y


def kernel(x, Wqkv, Wout):
    from concourse.bass_utils import run_bass_kernel_spmd

    if "nc" not in _COMPILED:
        _COMPILED["nc"] = _build_program()
    nc = _COMPILED["nc"]

    in_maps = _host_shard(x, Wqkv, Wout)
    res = run_bass_kernel_spmd(nc, in_maps, core_ids=list(range(NCORES)))
    _COMPILED["last_result"] = res
    return _host_assemble(res.results)


if __name__ == "__main__":
    # smoke build
    nc = _build_program()
    print("built ok; instructions:", len(nc.inst_map))


# revision 56
# speedup vs baseline: 1.0145x; 1.0145x over previous
"""Dilated multi-head attention (nn_DilatedMHA) on 8 trn2 NeuronCores.

Math (reference restructured):
  qkv = x @ Wqkv.T                      [b, n, 3, h, d]   b=2, n=8192, h=12, d=64
  Position i attends within its mod-2048 class {p, p+2048, p+4096, p+6144}
  (p = i % 2048).  Per group p and head: r=1 full 4x4 softmax; r=2 (p even)
  2x2 among same-parity slots; r=4 (p%4==0) adds v.  out is normalized by
  its sum over the sequence per (b, h*d) channel, then projected by Wout.

Sharding: core c <- batch c//4, groups p in [(c%4)*512, (c%4)*512+512).

Key precision structure: the normalization denominator s nearly cancels
(min |s| ~ 0.018 vs ~0.5 summands), so anything that feeds s is amplified
~1000x into the output.  Instead of summing the (possibly noisy) attention
output, s is computed on an exact side channel:
    s[h,d] = sum_p sum_jp C[p,h,jp] * (x[p,jp,:] @ Wv[:, (h,d)])
           = gT.T @ Wv-diag-blocks,   gT = sum_chunks x_chunk.T @ C_chunk
where C = sum_j Wt are the attention-weight column sums (fp32).  C needs
~2^-15-accurate scores; Q/K are projected as xh@wh in fp16 (1 cyc/col)
plus BOTH hi/lo cross terms in one fp8e5m2 DoubleRow matmul (0.5 cyc/col,
two K-slots per PE cell): slot0 = (xh/1024)@(wl*1024), slot1 =
(xl*32)@(wh/32) - per-operand scales cancel within each slot and keep the
tiny residuals inside e5m2's normal range.  Measured on hardware this adds
only ~1e-3 of output error (the numpy e5m2 emulation predicted 1.3e-2 -
the real PE is kinder than the emulation).

With s exact, every other path only needs ~12-bit relative accuracy and runs
at the PE's full 1 cycle/column fp16 rate: the V projection is a single fp16
matmul, AV/oacc/oT/out-projection are fp16 (2x DVE throughput, 1 cyc/col
transposes), and Wout is fp16 with 1/s folded in per channel.

Structure: pass 0 = Q,K 3-term projections + per-pair scores (Pool muls /
DVE reduces chase each chunk's PSUM evacuation) + softmax/Wt/C + g; the
s side channel + AllGather (1.875x cheaper than AllReduce) then overlap
pass 1 = V + AV + transposes + out-projection.  Block 3's g-matmuls are
deferred into pass 1 so their ~10us wait on C3 does not overflow the PE's
4-deep wait queue at the pass boundary.  PSUM start/stop flags ride on
full-tile zero-init matmuls because the tile scheduler freely reorders
per-slice accumulating matmuls (a slice-level `start` can execute late and
wipe earlier contributions).

Measured: rel err 1.9e-3 on hardware (budget 2e-2); cost-model device
time 219610 ns vs 415908 ns baseline (1.89x).  PE busy 179us of 220us
(81%): QK 92us (fp16 + fp8-DR), V 31us, out-proj 31us, g 15us,
transposes 5us.  Remaining idle: ~12us DMA head fill, ~15us collective
window (pass 1 drains before ws is ready), drain.  Known rejected
variants: DMA-XBAR transposes for oT, fp32 s-dot, AllReduce, sliced
weight-DMA priority, batched-by-jp scores in the last block - each
re-shuffles the tile scheduler into a worse global order (+2..15us).
A 3-term fp16 QK fallback (rel err 9.0e-4, 308505 ns) is kernel_308.py.
"""

import sys

if "/opt/trn_rl_repo" not in sys.path:
    sys.path.insert(0, "/opt/trn_rl_repo")

import numpy as np

EMBED = 768
HEADS = 12
HD = 64
B = 2
N = 8192
NCORES = 8
GPC = 512           # groups per core
NBLK = 4            # blocks of 128 groups per core
NCHUNK = 16         # row chunks of 128 per core (blk, j)
KO = 6              # embed // 128
QKW = 2 * EMBED     # Q|K output columns

_COMPILED = {}


def _build_program():
    import concourse.mybir as mybir
    import concourse.tile as tile
    from concourse import bacc

    F32 = mybir.dt.float32
    F16 = mybir.dt.float16
    F8 = mybir.dt.float8e5
    DR = mybir.MatmulPerfMode.DoubleRow
    AX = mybir.AxisListType
    OP = mybir.AluOpType
    ACTF = mybir.ActivationFunctionType

    nc = bacc.Bacc("TRN2", target_bir_lowering=False, debug=False, num_devices=NCORES)

    # --- DRAM I/O ---------------------------------------------------------
    xch_d = nc.dram_tensor("xch", [NCHUNK, 128, KO, 128], F16, kind="ExternalInput")
    x8_d = nc.dram_tensor("x8", [NCHUNK, 128, KO, 2, 128], F8, kind="ExternalInput")
    xrh_d = nc.dram_tensor("xrh", [NCHUNK, 128, KO, 128], F16, kind="ExternalInput")
    xrl_d = nc.dram_tensor("xrl", [NCHUNK, 128, KO, 128], F16, kind="ExternalInput")
    wqh_d = nc.dram_tensor("wqh", [128, KO, QKW], F16, kind="ExternalInput")
    wq8_d = nc.dram_tensor("wq8", [128, KO, 2, QKW], F8, kind="ExternalInput")
    wvh_d = nc.dram_tensor("wvh", [128, KO, EMBED], F16, kind="ExternalInput")
    wvl_d = nc.dram_tensor("wvl", [128, KO, EMBED], F16, kind="ExternalInput")
    wo_d = nc.dram_tensor("wo16", [128, KO, EMBED], F16, kind="ExternalInput")
    m2_d = nc.dram_tensor("m2", [128, 1], F32, kind="ExternalInput")
    m4_d = nc.dram_tensor("m4", [128, 1], F32, kind="ExternalInput")
    id_d = nc.dram_tensor("ident16", [128, 128], F16, kind="ExternalInput")
    y_d = nc.dram_tensor("y", [4 * GPC, EMBED], F32, kind="ExternalOutput")

    with tile.TileContext(nc) as tc:
        with (
            tc.tile_pool(name="const", bufs=1) as constp,
            tc.tile_pool(name="oT", bufs=4) as oTp,
            tc.tile_pool(name="wt16", bufs=4) as wt16p,
            tc.tile_pool(name="xch", bufs=NCHUNK) as xchp,
            tc.tile_pool(name="mm", bufs=3, space="PSUM") as mmp,
            tc.tile_pool(name="gp", bufs=1, space="PSUM") as gpp,
            tc.tile_pool(name="dram", bufs=2, space="DRAM") as dramp,
            tc.tile_pool(name="xr", bufs=6) as xrp,
            tc.tile_pool(name="att", bufs=1) as attp,
        ):
            # --- long-lived SBUF -----------------------------------------
            wqh_sb = constp.tile([128, KO, QKW], F16)
            wq8_sb = constp.tile([128, KO, 2, QKW], F8)
            wvh_sb = constp.tile([128, KO, EMBED], F16)
            wvl_sb = constp.tile([128, KO, EMBED], F16)
            wo_sb = constp.tile([128, KO, EMBED], F16)
            m2_sb = constp.tile([128, 1], F32)
            nc.sync.dma_start(m2_sb[:], m2_d[:])
            m4_sb = constp.tile([128, 1], F32)
            nc.sync.dma_start(m4_sb[:], m4_d[:])
            id_sb = constp.tile([128, 128], F16)
            nc.sync.dma_start(id_sb[:], id_d[:])
            zero384 = constp.tile([128, 384], F16)
            nc.vector.memset(zero384[:], 0.0)

            # gT accumulator [e-slice, ko, h]: gT = sum_chunks x_chunk.T @ C,
            # with the row-major xr tiles as the stationary operand and C as
            # the 12-column moving operand (N=12 -> ~1.4us of PE total), and
            # the output directly in s-dot layout.  A full-tile zero-init
            # matmul carries the single `start` so every accumulating matmul
            # has a data dependency on it (the scheduler is otherwise free to
            # reorder, which would run the start late and wipe contributions).
            gps = gpp.tile([128, KO, HEADS], F32, tag="g")
            nc.tensor.matmul(
                gps[:].rearrange("p a b -> p (a b)"), lhsT=id_sb[:],
                rhs=zero384[:, 0:KO * HEADS], start=True, stop=False,
            )

            oT_blocks = []
            wt16_blocks = []
            xch_tiles = {}
            deferred_g = []

            def emit_g(blk, xhl, Ch, Cl):
                for j in range(4):
                    _, _, xrh_sb, xrl_sb = xhl[j]
                    last = blk == NBLK - 1 and j == 3
                    chj = Ch[:, :, j:j + 1].rearrange("p h a -> p (h a)")
                    clj = Cl[:, :, j:j + 1].rearrange("p h a -> p (h a)")
                    for ko in range(KO):
                        nc.tensor.matmul(
                            gps[:, ko, :], lhsT=xrh_sb[:, ko, :], rhs=chj,
                            start=False, stop=False,
                        )
                        nc.tensor.matmul(
                            gps[:, ko, :], lhsT=xrh_sb[:, ko, :], rhs=clj,
                            start=False, stop=False,
                        )
                        nc.tensor.matmul(
                            gps[:, ko, :], lhsT=xrl_sb[:, ko, :], rhs=chj,
                            start=False, stop=(last and ko == KO - 1),
                        )

            def load_xch(chunk):
                t = xchp.tile([128, KO, 128], F16, tag="xch")
                nc.sync.dma_start(t[:], xch_d[chunk])
                xch_tiles[chunk] = t
                return t

            # ============ Pass 0: Q,K + scores + C + g + s ===============
            with (
                tc.tile_pool(name="xcl", bufs=3) as xclp,
                tc.tile_pool(name="qkv", bufs=2) as qkvp,
                tc.tile_pool(name="prs", bufs=1) as prsp,
            ):
                def load_xr(chunk):
                    xrh_sb = xrp.tile([128, KO, 128], F16, tag="xrh")
                    nc.sync.dma_start(xrh_sb[:], xrh_d[chunk])
                    xrl_sb = xrp.tile([128, KO, 128], F16, tag="xrl")
                    nc.sync.dma_start(xrl_sb[:], xrl_d[chunk])
                    return xrh_sb, xrl_sb

                def load_qk_x(chunk):
                    xh_sb = load_xch(chunk)
                    x8_sb = xclp.tile([128, KO, 2, 128], F8, tag="x8")
                    nc.sync.dma_start(x8_sb[:], x8_d[chunk])
                    return xh_sb, x8_sb

                def load_chunk(chunk):
                    return load_qk_x(chunk) + load_xr(chunk)

                # DMA priority: chunk-0 QK x, QK weights, chunk 1, xr later.
                h0 = load_qk_x(0)
                for ko in range(KO):
                    nc.sync.dma_start(wqh_sb[:, ko, :], wqh_d[:, ko, :])
                    nc.sync.dma_start(wq8_sb[:, ko, :, :], wq8_d[:, ko, :, :])
                h1 = load_qk_x(1)
                pre = {0: h0 + load_xr(0), 1: h1 + load_xr(1)}

                for blk in range(NBLK):
                    if blk == 2:
                        # pass-1 weights, off the critical path at both ends
                        nc.sync.dma_start(wvh_sb[:], wvh_d[:])
                        nc.sync.dma_start(wvl_sb[:], wvl_d[:])
                        nc.sync.dma_start(wo_sb[:], wo_d[:])
                    Qb = qkvp.tile([128, 4, EMBED], F32, tag="qb")
                    Kb = qkvp.tile([128, 4, EMBED], F32, tag="kb")
                    Q4 = Qb[:].rearrange("p j (h d) -> p j h d", d=HD)
                    K4 = Kb[:].rearrange("p j (h d) -> p j h d", d=HD)
                    S = attp.tile([128, 4, HEADS, 4], F32, tag="S")
                    xhl = []
                    npair = 0
                    for j in range(4):
                        chunk = blk * 4 + j
                        if chunk in pre:
                            tiles = pre.pop(chunk)
                        else:
                            tiles = load_chunk(chunk)
                        if chunk + 2 < NCHUNK and (chunk + 2) not in pre:
                            pre[chunk + 2] = load_chunk(chunk + 2)
                        xh_sb, x8_sb, xrh_sb, xrl_sb = tiles
                        xhl.append(tiles)
                        # Q|K: fp16 main term + fp8e5 DoubleRow cross terms
                        # (slot0 = (xh/1024)@(wl*1024), slot1 = (xl*32)@(wh/32))
                        for n in range(4):
                            ps = mmp.tile([128, 384], F32, tag="mm")
                            sl = slice(n * 384, (n + 1) * 384)
                            for ko in range(KO):
                                nc.tensor.matmul(
                                    ps[:], lhsT=xh_sb[:, ko, :],
                                    rhs=wqh_sb[:, ko, sl],
                                    start=(ko == 0), stop=False,
                                )
                                nc.tensor.matmul(
                                    ps[:], lhsT=x8_sb[:, ko, :, :],
                                    rhs=wq8_sb[:, ko, :, sl],
                                    perf_mode=DR,
                                    start=False, stop=(ko == KO - 1),
                                )
                            dest = Qb if n < 2 else Kb
                            nc.scalar.copy(
                                dest[:, j, (n % 2) * 384:(n % 2 + 1) * 384], ps[:]
                            )

                        # per-pair scores chase the chunk evacuations
                        pairs = [(b_, j) for b_ in range(j + 1)]
                        pairs += [(j, c_) for c_ in range(j)]
                        for b_, c_ in pairs:
                            pr = prsp.tile(
                                [128, HEADS, HD], F32, tag=f"prs{npair % 2}"
                            )
                            nc.gpsimd.tensor_mul(pr[:], Q4[:, b_], K4[:, c_])
                            nc.vector.reduce_sum(
                                S[:, b_, :, c_], pr[:], axis=AX.X
                            )
                            npair += 1

                    E = attp.tile([128, 4, HEADS, 4], F32, tag="E")
                    nc.scalar.activation(E[:], S[:], ACTF.Exp, scale=0.125)

                    # --- attention weights Wt (fp32) ---------------------
                    Z1 = attp.tile([128, 4, HEADS], F32, tag="Z1")
                    nc.vector.reduce_sum(Z1[:], E[:], axis=AX.X)
                    R1 = attp.tile([128, 4, HEADS], F32, tag="R1")
                    nc.vector.reciprocal(R1[:], Z1[:])
                    Z2 = attp.tile([128, 4, HEADS, 2], F32, tag="Z2")
                    nc.vector.tensor_add(Z2[:], E[:, :, :, 0:2], E[:, :, :, 2:4])
                    R2 = attp.tile([128, 4, HEADS, 2], F32, tag="R2")
                    nc.vector.reciprocal(R2[:], Z2[:])

                    W1 = attp.tile([128, 4, HEADS, 4], F32, tag="W1")
                    nc.vector.tensor_mul(
                        W1[:], E[:], R1[:, :, :, None].to_broadcast((128, 4, HEADS, 4))
                    )
                    W2 = attp.tile([128, 4, HEADS, 4], F32, tag="W2")
                    nc.vector.memset(W2[:], 0.0)
                    for par in (0, 1):
                        nc.vector.tensor_mul(
                            W2[:, par::2, :, par::2],
                            E[:, par::2, :, par::2],
                            R2[:, par::2, :, par:par + 1].to_broadcast(
                                (128, 2, HEADS, 2)
                            ),
                        )
                    Wt = attp.tile([128, 4, HEADS, 4], F32, tag="Wt")
                    nc.vector.scalar_tensor_tensor(
                        Wt[:], W2[:], m2_sb[:, 0:1], W1[:], OP.mult, OP.add
                    )
                    for j in range(4):
                        nc.vector.tensor_scalar_add(
                            Wt[:, j, :, j:j + 1], Wt[:, j, :, j:j + 1], m4_sb[:, 0:1]
                        )
                    Wt16 = wt16p.tile([128, 4, HEADS, 4], F16, tag="wt16")
                    nc.scalar.copy(Wt16[:], Wt[:])
                    wt16_blocks.append(Wt16)

                    # --- C = column sums of Wt (exact, fp32) -------------
                    C = attp.tile([128, HEADS, 4], F32, tag="C")
                    nc.vector.reduce_sum(
                        C[:], Wt[:].rearrange("p j h k -> p h k j"), axis=AX.X
                    )
                    Ch = attp.tile([128, HEADS, 4], F16, tag="Ch")
                    nc.scalar.copy(Ch[:], C[:])
                    Cl = attp.tile([128, HEADS, 4], F16, tag="Cl")
                    nc.vector.tensor_sub(Cl[:], C[:], Ch[:])

                    # --- g[h, e] += C_chunk[p, h]^T @ x_chunk[p, e] -------
                    # 3-term fp16; xr tiles are row-major [p, (ko, e)].
                    # Block 3's g-matmuls wait ~10us on its C; deferring them
                    # into pass 1 keeps the PE wait queue from blocking the
                    # pass boundary.
                    if blk < NBLK - 1:
                        emit_g(blk, xhl, Ch, Cl)
                    else:
                        deferred_g.append((blk, xhl, Ch, Cl))

            # =============== exact s + AllGather (emitted mid pass 1) ====
            def emit_s_tail():
                for args in deferred_g:
                    emit_g(*args)
                g_sb = constp.tile([128, KO, HEADS], F32)
                nc.scalar.copy(g_sb[:], gps[:])
                gh = constp.tile([128, KO, HEADS], F16)
                nc.scalar.copy(gh[:], g_sb[:])
                gl = constp.tile([128, KO, HEADS], F16)
                nc.vector.tensor_sub(gl[:], g_sb[:], gh[:])

                # s matmuls accumulate into a gp-pool tile (tag reuse keeps
                # PSUM within 8 banks)
                stp = gpp.tile([128, KO, 2], F32, tag="gt")
                nc.tensor.matmul(
                    stp[:].rearrange("p a b -> p (a b)"), lhsT=id_sb[:],
                    rhs=zero384[:, 0:KO * 2], start=True, stop=False,
                )
                for t in range(KO):
                    sl = slice(t * 128, (t + 1) * 128)
                    hs = slice(2 * t, 2 * t + 2)
                    for ko in range(KO):
                        nc.tensor.matmul(
                            stp[:, t, :], lhsT=wvh_sb[:, ko, sl],
                            rhs=gh[:, ko, hs], start=False, stop=False,
                        )
                        nc.tensor.matmul(
                            stp[:, t, :], lhsT=wvh_sb[:, ko, sl],
                            rhs=gl[:, ko, hs], start=False, stop=False,
                        )
                        nc.tensor.matmul(
                            stp[:, t, :], lhsT=wvl_sb[:, ko, sl],
                            rhs=gh[:, ko, hs], start=False,
                            stop=(t == KO - 1 and ko == KO - 1),
                        )
                s_chan = constp.tile([128, KO], F32)
                for t in range(KO):
                    nc.vector.tensor_copy(s_chan[0:64, t:t + 1], stp[0:64, t, 0:1])
                    nc.vector.tensor_copy(
                        s_chan[64:128, t:t + 1], stp[64:128, t, 1:2]
                    )

                # AllGather (1.875x cheaper than AllReduce in latency) of the
                # four quarter-core partial sums, then add locally.
                cc_in = dramp.tile([128, KO], F32)
                cc_out = dramp.tile([4, 128, KO], F32)
                nc.gpsimd.dma_start(cc_in[:], s_chan[:])
                nc.gpsimd.collective_compute(
                    "AllGather",
                    OP.bypass,
                    replica_groups=[[0, 1, 2, 3], [4, 5, 6, 7]],
                    ins=[cc_in[:].opt()],
                    outs=[cc_out[:].opt()],
                )
                s_gath = constp.tile([128, 4, KO], F32)
                nc.gpsimd.dma_start(
                    s_gath[:], cc_out[:].rearrange("g p t -> p g t")
                )
                sa = constp.tile([128, KO], F32)
                nc.vector.tensor_add(sa[:], s_gath[:, 0, :], s_gath[:, 1, :])
                sb2 = constp.tile([128, KO], F32)
                nc.vector.tensor_add(sb2[:], s_gath[:, 2, :], s_gath[:, 3, :])
                s_tot = constp.tile([128, KO], F32)
                nc.vector.tensor_add(s_tot[:], sa[:], sb2[:])
                r_sb = constp.tile([128, KO], F32)
                nc.vector.reciprocal(r_sb[:], s_tot[:])
                return r_sb

            # ====== Pass 1: V + AV + transposes (collective hidden) ======
            with (
                tc.tile_pool(name="vq", bufs=2) as vqp,
                tc.tile_pool(name="avt", bufs=2) as avtp,
                tc.tile_pool(name="oacc", bufs=2) as oaccp,
                tc.tile_pool(name="ws", bufs=1) as wsp,
                tc.tile_pool(name="fin", bufs=2) as finp,
                tc.tile_pool(name="tp", bufs=2, space="PSUM") as tpp,
            ):
                r_sb = None
                for blk in range(NBLK):
                    if blk == 1:
                        r_sb = emit_s_tail()
                    V16 = vqp.tile([128, 4, EMBED], F16, tag="vb")
                    for j in range(4):
                        xh_sb = xch_tiles[blk * 4 + j]
                        for n in range(2):
                            ps = mmp.tile([128, 384], F32, tag="mm")
                            sl = slice(n * 384, (n + 1) * 384)
                            for ko in range(KO):
                                nc.tensor.matmul(
                                    ps[:], lhsT=xh_sb[:, ko, :],
                                    rhs=wvh_sb[:, ko, sl],
                                    start=(ko == 0), stop=(ko == KO - 1),
                                )
                            nc.scalar.copy(V16[:, j, n * 384:(n + 1) * 384], ps[:])

                    # --- AV (fp16): oacc[p, j] = sum_jp Wt16 * V16 -------
                    # muls on Pool (flat 0.833ns/elem, broadcast-immune),
                    # adds on DVE (fp16 packed 2x)
                    Wt16 = wt16_blocks[blk]
                    oacc = oaccp.tile([128, 4, EMBED], F16, tag="oacc")
                    o4 = oacc[:].rearrange("p j (h d) -> p j h d", d=HD)
                    for jp in range(4):
                        vb = (
                            V16[:, jp:jp + 1, :]
                            .rearrange("p a (h d) -> p a h d", d=HD)
                            .to_broadcast((128, 4, HEADS, HD))
                        )
                        wb = Wt16[:, :, :, jp:jp + 1].to_broadcast(
                            (128, 4, HEADS, HD)
                        )
                        if jp == 0:
                            nc.vector.scalar_tensor_tensor(
                                o4[:], vb, 1.0, wb, OP.mult, OP.mult
                            )
                        else:
                            t = avtp.tile(
                                [128, 4, HEADS, HD], F16, tag=f"avt{jp % 2}"
                            )
                            nc.gpsimd.tensor_mul(t[:], vb, wb)
                            nc.vector.tensor_add(o4[:], o4[:], t[:])

                    # --- transpose oacc -> oT[hd, rows] (fp16, 1 cyc) ----
                    oT = oTp.tile([128, KO, 4 * 128], F16, tag="oT")
                    for j in range(4):
                        for ko in range(KO):
                            pt = tpp.tile([128, 128], F16, tag="tp")
                            nc.tensor.transpose(
                                pt[:], oacc[:, j, ko * 128:(ko + 1) * 128], id_sb[:]
                            )
                            if (j * KO + ko) % 3 == 0:
                                nc.vector.tensor_copy(
                                    oT[:, ko, j * 128:(j + 1) * 128], pt[:]
                                )
                            else:
                                nc.scalar.copy(
                                    oT[:, ko, j * 128:(j + 1) * 128], pt[:]
                                )
                    oT_blocks.append(oT)

                # =============== out-projection ==========================
                ws_sb = wsp.tile([128, KO, EMBED], F16)
                for ko in range(KO):
                    nc.vector.tensor_scalar_mul(
                        ws_sb[:, ko, :], wo_sb[:, ko, :], r_sb[:, ko:ko + 1]
                    )

                for blk in range(NBLK):
                    oT = oT_blocks[blk]
                    for rc in range(4):
                        for half in range(2):
                            pf = mmp.tile([128, 384], F32, tag="mm")
                            for ko in range(KO):
                                nc.tensor.matmul(
                                    pf[:],
                                    lhsT=oT[:, ko, rc * 128:(rc + 1) * 128],
                                    rhs=ws_sb[:, ko, half * 384:(half + 1) * 384],
                                    start=(ko == 0),
                                    stop=(ko == KO - 1),
                                )
                            fin = finp.tile([128, 384], F32, tag="fin")
                            nc.scalar.copy(fin[:], pf[:])
                            rows = blk * 512 + rc * 128
                            nc.sync.dma_start(
                                y_d[rows:rows + 128, half * 384:(half + 1) * 384],
                                fin[:],
                            )

    nc.finalize()
    return nc


def _host_shard(x, Wqkv, Wout):
    """Build per-core input maps."""
    x = np.ascontiguousarray(np.asarray(x, dtype=np.float32))
    Wqkv = np.asarray(Wqkv, dtype=np.float32)
    Wout = np.asarray(Wout, dtype=np.float32)

    wq = np.ascontiguousarray(
        Wqkv.T.reshape(KO, 128, 3 * EMBED).transpose(1, 0, 2)
    )
    import concourse.mybir as _mybir
    F8NP = _mybir.dt.np(_mybir.dt.float8e5)
    wqk = wq[:, :, :QKW]
    wqh = np.ascontiguousarray(wqk.astype(np.float16))
    wql32 = wqk - wqh.astype(np.float32)
    # wq8[:, ko, 0, :] = wl*1024 (pairs with xh/1024); [:, ko, 1, :] = wh/32
    wq8 = np.empty((128, KO, 2, QKW), dtype=F8NP)
    wq8[:, :, 0, :] = (wql32 * 1024.0).astype(F8NP)
    wq8[:, :, 1, :] = (wqh.astype(np.float32) / 32.0).astype(F8NP)
    wv = wq[:, :, QKW:]
    wvh = np.ascontiguousarray(wv.astype(np.float16))
    wvl = np.ascontiguousarray((wv - wvh.astype(np.float32)).astype(np.float16))
    wo16 = np.ascontiguousarray(
        Wout.T.reshape(KO, 128, EMBED).transpose(1, 0, 2).astype(np.float16)
    )
    m2 = (np.arange(128) % 2 == 0).astype(np.float32).reshape(128, 1)
    m4 = (np.arange(128) % 4 == 0).astype(np.float32).reshape(128, 1)
    ident16 = np.eye(128, dtype=np.float16)

    in_maps = []
    for c in range(NCORES):
        bc, q = divmod(c, 4)
        xb = x[bc].reshape(4, 4, 4, 128, EMBED)  # [j, q, blk, g, e]
        mine = xb[:, q]                          # [j, blk, g, e]
        t = np.ascontiguousarray(mine.transpose(1, 0, 2, 3)).reshape(
            NCHUNK, 128, EMBED
        )
        xc = np.ascontiguousarray(
            t.reshape(NCHUNK, 128, KO, 128).transpose(0, 3, 2, 1)
        )
        xch = xc.astype(np.float16)
        xcl32 = xc - xch.astype(np.float32)
        x8 = np.empty((NCHUNK, 128, KO, 2, 128), dtype=F8NP)
        x8[:, :, :, 0, :] = (xch.astype(np.float32) / 1024.0).astype(F8NP)
        x8[:, :, :, 1, :] = (xcl32 * 32.0).astype(F8NP)
        xr = np.ascontiguousarray(t.reshape(NCHUNK, 128, KO, 128))
        xrh = xr.astype(np.float16)
        xrl = (xr - xrh.astype(np.float32)).astype(np.float16)
        in_maps.append(
            {
                "xch": xch, "x8": x8, "xrh": xrh, "xrl": xrl,
                "wqh": wqh, "wq8": wq8, "wvh": wvh, "wvl": wvl,
                "wo16": wo16, "m2": m2, "m4": m4, "ident16": ident16,
            }
        )
    return in_maps


def _host_assemble(results):
    y = np.empty((B, N, EMBED), dtype=np.float32)
    for c in range(NCORES):
        bc, q = divmod(c, 4)
        yc = np.asarray(results[c]["y"])  # [2048, 768], rows (blk, j, g)
        part = yc.reshape(4, 4, 128, EMBED).transpose(1, 0, 2, 3)  # [j, blk, g, e]
        y[bc].reshape(4, 4, 4, 128, EMBED)[:, q] = part
    return y


def kernel(x, Wqkv, Wout):
    from concourse.bass_utils import run_bass_kernel_spmd

    if "nc" not in _COMPILED:
        _COMPILED["nc"] = _build_program()
    nc = _COMPILED["nc"]

    in_maps = _host_shard(x, Wqkv, Wout)
    res = run_bass_kernel_spmd(nc, in_maps, core_ids=list(range(NCORES)))
    _COMPILED["last_result"] = res
    return _host_assemble(res.results)


if __name__ == "__main__":
    # smoke build
    nc = _build_program()
    print("built ok; instructions:", len(nc.inst_map))


# revision 57
# speedup vs baseline: 1.0937x; 1.0781x over previous
"""Dilated multi-head attention (nn_DilatedMHA) on 8 trn2 NeuronCores.

Math (reference restructured):
  qkv = x @ Wqkv.T                      [b, n, 3, h, d]   b=2, n=8192, h=12, d=64
  Position i attends within its mod-2048 class {p, p+2048, p+4096, p+6144}
  (p = i % 2048).  Per group p and head: r=1 full 4x4 softmax; r=2 (p even)
  2x2 among same-parity slots; r=4 (p%4==0) adds v.  out is normalized by
  its sum over the sequence per (b, h*d) channel, then projected by Wout.

Sharding: core c <- batch c//4, groups p in [(c%4)*512, (c%4)*512+512).

Key precision structure: the normalization denominator s nearly cancels
(min |s| ~ 0.018 vs ~0.5 summands), so anything that feeds s is amplified
~1000x into the output.  Instead of summing the (possibly noisy) attention
output, s is computed on an exact side channel:
    s[h,d] = sum_p sum_jp C[p,h,jp] * (x[p,jp,:] @ Wv[:, (h,d)])
           = gT.T @ Wv-diag-blocks,   gT = sum_chunks x_chunk.T @ C_chunk
where C = sum_j Wt are the attention-weight column sums (fp32).  C needs
~2^-15-accurate scores; Q/K are projected as xh@wh in fp16 (1 cyc/col)
plus BOTH hi/lo cross terms in one fp8e5m2 DoubleRow matmul (0.5 cyc/col,
two K-slots per PE cell): slot0 = (xh/1024)@(wl*1024), slot1 =
(xl*32)@(wh/32) - per-operand scales cancel within each slot and keep the
tiny residuals inside e5m2's normal range.  Measured on hardware this adds
only ~1e-3 of output error (the numpy e5m2 emulation predicted 1.3e-2 -
the real PE is kinder than the emulation).

With s exact, every other path only needs ~12-bit relative accuracy and runs
at the PE's full 1 cycle/column fp16 rate: the V projection is a single fp16
matmul, AV/oacc/oT/out-projection are fp16 (2x DVE throughput, 1 cyc/col
transposes), and Wout is fp16 with 1/s folded in per channel.

Structure: pass 0 = Q,K 3-term projections + per-pair scores (Pool muls /
DVE reduces chase each chunk's PSUM evacuation) + softmax/Wt/C + g; the
s side channel + AllGather (1.875x cheaper than AllReduce) then overlap
pass 1 = V + AV + transposes + out-projection.  Block 3's g-matmuls are
deferred into pass 1 so their ~10us wait on C3 does not overflow the PE's
4-deep wait queue at the pass boundary.  PSUM start/stop flags ride on
full-tile zero-init matmuls because the tile scheduler freely reorders
per-slice accumulating matmuls (a slice-level `start` can execute late and
wipe earlier contributions).

Measured: rel err 1.9e-3 on hardware (budget 2e-2); cost-model device
time 219610 ns vs 415908 ns baseline (1.89x).  PE busy 179us of 220us
(81%): QK 92us (fp16 + fp8-DR), V 31us, out-proj 31us, g 15us,
transposes 5us.  Remaining idle: ~12us DMA head fill, ~15us collective
window (pass 1 drains before ws is ready), drain.  Known rejected
variants: DMA-XBAR transposes for oT, fp32 s-dot, AllReduce, sliced
weight-DMA priority, batched-by-jp scores in the last block - each
re-shuffles the tile scheduler into a worse global order (+2..15us).
A 3-term fp16 QK fallback (rel err 9.0e-4, 308505 ns) is kernel_308.py.
"""

import sys

if "/opt/trn_rl_repo" not in sys.path:
    sys.path.insert(0, "/opt/trn_rl_repo")

import numpy as np

EMBED = 768
HEADS = 12
HD = 64
B = 2
N = 8192
NCORES = 8
GPC = 512           # groups per core
NBLK = 4            # blocks of 128 groups per core
NCHUNK = 16         # row chunks of 128 per core (blk, j)
KO = 6              # embed // 128
QKW = 2 * EMBED     # Q|K output columns

_COMPILED = {}


def _build_program():
    import concourse.mybir as mybir
    import concourse.tile as tile
    from concourse import bacc

    F32 = mybir.dt.float32
    F16 = mybir.dt.float16
    F8 = mybir.dt.float8e5
    DR = mybir.MatmulPerfMode.DoubleRow
    AX = mybir.AxisListType
    OP = mybir.AluOpType
    ACTF = mybir.ActivationFunctionType

    nc = bacc.Bacc("TRN2", target_bir_lowering=False, debug=False, num_devices=NCORES)

    # --- DRAM I/O ---------------------------------------------------------
    xch_d = nc.dram_tensor("xch", [NCHUNK, 128, KO, 128], F16, kind="ExternalInput")
    x8_d = nc.dram_tensor("x8", [NCHUNK, 128, KO, 2, 128], F8, kind="ExternalInput")
    xrh_d = nc.dram_tensor("xrh", [NCHUNK, 128, KO, 128], F16, kind="ExternalInput")
    xrl_d = nc.dram_tensor("xrl", [NCHUNK, 128, KO, 128], F16, kind="ExternalInput")
    wqh_d = nc.dram_tensor("wqh", [128, KO, QKW], F16, kind="ExternalInput")
    wq8_d = nc.dram_tensor("wq8", [128, KO, 2, QKW], F8, kind="ExternalInput")
    wvh_d = nc.dram_tensor("wvh", [128, KO, EMBED], F16, kind="ExternalInput")
    wvl_d = nc.dram_tensor("wvl", [128, KO, EMBED], F16, kind="ExternalInput")
    wo_d = nc.dram_tensor("wo16", [128, KO, EMBED], F16, kind="ExternalInput")
    m2_d = nc.dram_tensor("m2", [128, 1], F32, kind="ExternalInput")
    m4_d = nc.dram_tensor("m4", [128, 1], F32, kind="ExternalInput")
    id_d = nc.dram_tensor("ident16", [128, 128], F16, kind="ExternalInput")
    y_d = nc.dram_tensor("y", [4 * GPC, EMBED], F32, kind="ExternalOutput")

    with tile.TileContext(nc) as tc:
        with (
            tc.tile_pool(name="const", bufs=1) as constp,
            tc.tile_pool(name="oT", bufs=4) as oTp,
            tc.tile_pool(name="wt16", bufs=4) as wt16p,
            tc.tile_pool(name="xch", bufs=NCHUNK) as xchp,
            tc.tile_pool(name="mm", bufs=3, space="PSUM") as mmp,
            tc.tile_pool(name="gp", bufs=1, space="PSUM") as gpp,
            tc.tile_pool(name="dram", bufs=2, space="DRAM") as dramp,
            tc.tile_pool(name="xr", bufs=6) as xrp,
            tc.tile_pool(name="att", bufs=1) as attp,
        ):
            # --- long-lived SBUF -----------------------------------------
            wqh_sb = constp.tile([128, KO, QKW], F16)
            wq8_sb = constp.tile([128, KO, 2, QKW], F8)
            wvh_sb = constp.tile([128, KO, EMBED], F16)
            wvl_sb = constp.tile([128, KO, EMBED], F16)
            wo_sb = constp.tile([128, KO, EMBED], F16)
            m2_sb = constp.tile([128, 1], F32)
            nc.sync.dma_start(m2_sb[:], m2_d[:])
            m4_sb = constp.tile([128, 1], F32)
            nc.sync.dma_start(m4_sb[:], m4_d[:])
            id_sb = constp.tile([128, 128], F16)
            nc.sync.dma_start(id_sb[:], id_d[:])
            zero384 = constp.tile([128, 384], F16)
            nc.vector.memset(zero384[:], 0.0)

            # gT accumulator [e-slice, ko, h]: gT = sum_chunks x_chunk.T @ C,
            # with the row-major xr tiles as the stationary operand and C as
            # the 12-column moving operand (N=12 -> ~1.4us of PE total), and
            # the output directly in s-dot layout.  A full-tile zero-init
            # matmul carries the single `start` so every accumulating matmul
            # has a data dependency on it (the scheduler is otherwise free to
            # reorder, which would run the start late and wipe contributions).
            gps = gpp.tile([128, KO, HEADS], F32, tag="g")
            nc.tensor.matmul(
                gps[:].rearrange("p a b -> p (a b)"), lhsT=id_sb[:],
                rhs=zero384[:, 0:KO * HEADS], start=True, stop=False,
            )

            oT_blocks = []
            wt16_blocks = []
            xch_tiles = {}
            deferred_g = []

            def emit_g(blk, xhl, Ch, Cl):
                for j in range(4):
                    _, _, xrh_sb, xrl_sb = xhl[j]
                    last = blk == NBLK - 1 and j == 3
                    chj = Ch[:, :, j:j + 1].rearrange("p h a -> p (h a)")
                    clj = Cl[:, :, j:j + 1].rearrange("p h a -> p (h a)")
                    for ko in range(KO):
                        nc.tensor.matmul(
                            gps[:, ko, :], lhsT=xrh_sb[:, ko, :], rhs=chj,
                            start=False, stop=False,
                        )
                        nc.tensor.matmul(
                            gps[:, ko, :], lhsT=xrh_sb[:, ko, :], rhs=clj,
                            start=False, stop=False,
                        )
                        nc.tensor.matmul(
                            gps[:, ko, :], lhsT=xrl_sb[:, ko, :], rhs=chj,
                            start=False, stop=(last and ko == KO - 1),
                        )

            def load_xch(chunk):
                t = xchp.tile([128, KO, 128], F16, tag="xch")
                nc.sync.dma_start(t[:], xch_d[chunk])
                xch_tiles[chunk] = t
                return t

            # ============ Pass 0: Q,K + scores + C + g + s ===============
            with (
                tc.tile_pool(name="xcl", bufs=3) as xclp,
                tc.tile_pool(name="qkv", bufs=2) as qkvp,
                tc.tile_pool(name="prs", bufs=1) as prsp,
            ):
                def load_xr(chunk):
                    xrh_sb = xrp.tile([128, KO, 128], F16, tag="xrh")
                    nc.sync.dma_start(xrh_sb[:], xrh_d[chunk])
                    xrl_sb = xrp.tile([128, KO, 128], F16, tag="xrl")
                    nc.sync.dma_start(xrl_sb[:], xrl_d[chunk])
                    return xrh_sb, xrl_sb

                def load_qk_x(chunk):
                    xh_sb = load_xch(chunk)
                    x8_sb = xclp.tile([128, KO, 2, 128], F8, tag="x8")
                    nc.sync.dma_start(x8_sb[:], x8_d[chunk])
                    return xh_sb, x8_sb

                def load_chunk(chunk):
                    return load_qk_x(chunk) + load_xr(chunk)

                # DMA priority: chunk-0 QK x, QK weights, chunk 1, xr later.
                h0 = load_qk_x(0)
                for ko in range(KO):
                    nc.sync.dma_start(wqh_sb[:, ko, :], wqh_d[:, ko, :])
                    nc.sync.dma_start(wq8_sb[:, ko, :, :], wq8_d[:, ko, :, :])
                h1 = load_qk_x(1)
                pre = {0: h0 + load_xr(0), 1: h1 + load_xr(1)}

                for blk in range(NBLK):
                    if blk == 2:
                        # pass-1 weights, off the critical path at both ends
                        nc.sync.dma_start(wvh_sb[:], wvh_d[:])
                        nc.sync.dma_start(wvl_sb[:], wvl_d[:])
                        nc.sync.dma_start(wo_sb[:], wo_d[:])
                    Qb = qkvp.tile([128, 4, EMBED], F32, tag="qb")
                    Kb = qkvp.tile([128, 4, EMBED], F32, tag="kb")
                    Q4 = Qb[:].rearrange("p j (h d) -> p j h d", d=HD)
                    K4 = Kb[:].rearrange("p j (h d) -> p j h d", d=HD)
                    S = attp.tile([128, 4, HEADS, 4], F32, tag="S")
                    xhl = []
                    npair = 0
                    for j in range(4):
                        chunk = blk * 4 + j
                        if chunk in pre:
                            tiles = pre.pop(chunk)
                        else:
                            tiles = load_chunk(chunk)
                        if chunk + 2 < NCHUNK and (chunk + 2) not in pre:
                            pre[chunk + 2] = load_chunk(chunk + 2)
                        xh_sb, x8_sb, xrh_sb, xrl_sb = tiles
                        xhl.append(tiles)
                        # Q|K: fp16 main term + fp8e5 DoubleRow cross terms
                        # (slot0 = (xh/1024)@(wl*1024), slot1 = (xl*32)@(wh/32))
                        for n in range(4):
                            ps = mmp.tile([128, 384], F32, tag="mm")
                            sl = slice(n * 384, (n + 1) * 384)
                            for ko in range(KO):
                                nc.tensor.matmul(
                                    ps[:], lhsT=xh_sb[:, ko, :],
                                    rhs=wqh_sb[:, ko, sl],
                                    start=(ko == 0), stop=False,
                                )
                                nc.tensor.matmul(
                                    ps[:], lhsT=x8_sb[:, ko, :, :],
                                    rhs=wq8_sb[:, ko, :, sl],
                                    perf_mode=DR,
                                    start=False, stop=(ko == KO - 1),
                                )
                            dest = Qb if n < 2 else Kb
                            nc.scalar.copy(
                                dest[:, j, (n % 2) * 384:(n % 2 + 1) * 384], ps[:]
                            )

                        # per-pair scores chase the chunk evacuations
                        pairs = [(b_, j) for b_ in range(j + 1)]
                        pairs += [(j, c_) for c_ in range(j)]
                        for b_, c_ in pairs:
                            pr = prsp.tile(
                                [128, HEADS, HD], F32, tag=f"prs{npair % 2}"
                            )
                            nc.gpsimd.tensor_mul(pr[:], Q4[:, b_], K4[:, c_])
                            nc.vector.reduce_sum(
                                S[:, b_, :, c_], pr[:], axis=AX.X
                            )
                            npair += 1

                    E = attp.tile([128, 4, HEADS, 4], F32, tag="E")
                    nc.scalar.activation(E[:], S[:], ACTF.Exp, scale=0.125)

                    # --- attention weights Wt (fp32) ---------------------
                    Z1 = attp.tile([128, 4, HEADS], F32, tag="Z1")
                    nc.vector.reduce_sum(Z1[:], E[:], axis=AX.X)
                    R1 = attp.tile([128, 4, HEADS], F32, tag="R1")
                    nc.vector.reciprocal(R1[:], Z1[:])
                    Z2 = attp.tile([128, 4, HEADS, 2], F32, tag="Z2")
                    nc.vector.tensor_add(Z2[:], E[:, :, :, 0:2], E[:, :, :, 2:4])
                    R2 = attp.tile([128, 4, HEADS, 2], F32, tag="R2")
                    nc.vector.reciprocal(R2[:], Z2[:])

                    W1 = attp.tile([128, 4, HEADS, 4], F32, tag="W1")
                    nc.vector.tensor_mul(
                        W1[:], E[:], R1[:, :, :, None].to_broadcast((128, 4, HEADS, 4))
                    )
                    W2 = attp.tile([128, 4, HEADS, 4], F32, tag="W2")
                    nc.vector.memset(W2[:], 0.0)
                    for par in (0, 1):
                        nc.vector.tensor_mul(
                            W2[:, par::2, :, par::2],
                            E[:, par::2, :, par::2],
                            R2[:, par::2, :, par:par + 1].to_broadcast(
                                (128, 2, HEADS, 2)
                            ),
                        )
                    Wt = attp.tile([128, 4, HEADS, 4], F32, tag="Wt")
                    nc.vector.scalar_tensor_tensor(
                        Wt[:], W2[:], m2_sb[:, 0:1], W1[:], OP.mult, OP.add
                    )
                    for j in range(4):
                        nc.vector.tensor_scalar_add(
                            Wt[:, j, :, j:j + 1], Wt[:, j, :, j:j + 1], m4_sb[:, 0:1]
                        )
                    Wt16 = wt16p.tile([128, 4, HEADS, 4], F16, tag="wt16")
                    nc.scalar.copy(Wt16[:], Wt[:])
                    wt16_blocks.append(Wt16)

                    # --- C = column sums of Wt (exact, fp32) -------------
                    C = attp.tile([128, HEADS, 4], F32, tag="C")
                    nc.vector.reduce_sum(
                        C[:], Wt[:].rearrange("p j h k -> p h k j"), axis=AX.X
                    )
                    Ch = attp.tile([128, HEADS, 4], F16, tag="Ch")
                    nc.scalar.copy(Ch[:], C[:])
                    Cl = attp.tile([128, HEADS, 4], F16, tag="Cl")
                    nc.vector.tensor_sub(Cl[:], C[:], Ch[:])

                    # --- g[h, e] += C_chunk[p, h]^T @ x_chunk[p, e] -------
                    # 3-term fp16; xr tiles are row-major [p, (ko, e)].
                    # Block 3's g-matmuls wait ~10us on its C; deferring them
                    # into pass 1 keeps the PE wait queue from blocking the
                    # pass boundary.
                    if blk < NBLK - 1:
                        emit_g(blk, xhl, Ch, Cl)
                    else:
                        deferred_g.append((blk, xhl, Ch, Cl))

            # =============== exact s + AllGather (emitted mid pass 1) ====
            def emit_s_tail():
                for args in deferred_g:
                    emit_g(*args)
                g_sb = constp.tile([128, KO, HEADS], F32)
                nc.scalar.copy(g_sb[:], gps[:])
                gh = constp.tile([128, KO, HEADS], F16)
                nc.scalar.copy(gh[:], g_sb[:])
                gl = constp.tile([128, KO, HEADS], F16)
                nc.vector.tensor_sub(gl[:], g_sb[:], gh[:])

                # s matmuls accumulate into a gp-pool tile (tag reuse keeps
                # PSUM within 8 banks)
                stp = gpp.tile([128, KO, 2], F32, tag="gt")
                nc.tensor.matmul(
                    stp[:].rearrange("p a b -> p (a b)"), lhsT=id_sb[:],
                    rhs=zero384[:, 0:KO * 2], start=True, stop=False,
                )
                for t in range(KO):
                    sl = slice(t * 128, (t + 1) * 128)
                    hs = slice(2 * t, 2 * t + 2)
                    for ko in range(KO):
                        nc.tensor.matmul(
                            stp[:, t, :], lhsT=wvh_sb[:, ko, sl],
                            rhs=gh[:, ko, hs], start=False, stop=False,
                        )
                        nc.tensor.matmul(
                            stp[:, t, :], lhsT=wvh_sb[:, ko, sl],
                            rhs=gl[:, ko, hs], start=False, stop=False,
                        )
                        nc.tensor.matmul(
                            stp[:, t, :], lhsT=wvl_sb[:, ko, sl],
                            rhs=gh[:, ko, hs], start=False,
                            stop=(t == KO - 1 and ko == KO - 1),
                        )
                s_chan = constp.tile([128, KO], F32)
                for t in range(KO):
                    nc.vector.tensor_copy(s_chan[0:64, t:t + 1], stp[0:64, t, 0:1])
                    nc.vector.tensor_copy(
                        s_chan[64:128, t:t + 1], stp[64:128, t, 1:2]
                    )

                # AllGather (1.875x cheaper than AllReduce in latency) of the
                # four quarter-core partial sums, then add locally.
                cc_in = dramp.tile([128, KO], F32)
                cc_out = dramp.tile([4, 128, KO], F32)
                nc.gpsimd.dma_start(cc_in[:], s_chan[:])
                nc.gpsimd.collective_compute(
                    "AllGather",
                    OP.bypass,
                    replica_groups=[[0, 1, 2, 3], [4, 5, 6, 7]],
                    ins=[cc_in[:].opt()],
                    outs=[cc_out[:].opt()],
                )
                s_gath = constp.tile([128, 4, KO], F32)
                nc.gpsimd.dma_start(
                    s_gath[:], cc_out[:].rearrange("g p t -> p g t")
                )
                sa = constp.tile([128, KO], F32)
                nc.vector.tensor_add(sa[:], s_gath[:, 0, :], s_gath[:, 1, :])
                sb2 = constp.tile([128, KO], F32)
                nc.vector.tensor_add(sb2[:], s_gath[:, 2, :], s_gath[:, 3, :])
                s_tot = constp.tile([128, KO], F32)
                nc.vector.tensor_add(s_tot[:], sa[:], sb2[:])
                r_sb = constp.tile([128, KO], F32)
                nc.vector.reciprocal(r_sb[:], s_tot[:])
                return r_sb

            # ====== Pass 1: V + AV + transposes (collective hidden) ======
            with (
                tc.tile_pool(name="vq", bufs=2) as vqp,
                tc.tile_pool(name="avt", bufs=2) as avtp,
                tc.tile_pool(name="oacc", bufs=2) as oaccp,
                tc.tile_pool(name="ws", bufs=1) as wsp,
                tc.tile_pool(name="fin", bufs=4) as finp,
                tc.tile_pool(name="tp", bufs=2, space="PSUM") as tpp,
            ):
                r_sb = None
                for blk in range(NBLK):
                    if blk == 1:
                        r_sb = emit_s_tail()
                    V16 = vqp.tile([128, 4, EMBED], F16, tag="vb")
                    for j in range(4):
                        xh_sb = xch_tiles[blk * 4 + j]
                        for n in range(2):
                            ps = mmp.tile([128, 384], F32, tag="mm")
                            sl = slice(n * 384, (n + 1) * 384)
                            for ko in range(KO):
                                nc.tensor.matmul(
                                    ps[:], lhsT=xh_sb[:, ko, :],
                                    rhs=wvh_sb[:, ko, sl],
                                    start=(ko == 0), stop=(ko == KO - 1),
                                )
                            nc.scalar.copy(V16[:, j, n * 384:(n + 1) * 384], ps[:])

                    # --- AV (fp16): oacc[p, j] = sum_jp Wt16 * V16 -------
                    # muls on Pool (flat 0.833ns/elem, broadcast-immune),
                    # adds on DVE (fp16 packed 2x)
                    Wt16 = wt16_blocks[blk]
                    oacc = oaccp.tile([128, 4, EMBED], F16, tag="oacc")
                    o4 = oacc[:].rearrange("p j (h d) -> p j h d", d=HD)
                    for jp in range(4):
                        vb = (
                            V16[:, jp:jp + 1, :]
                            .rearrange("p a (h d) -> p a h d", d=HD)
                            .to_broadcast((128, 4, HEADS, HD))
                        )
                        wb = Wt16[:, :, :, jp:jp + 1].to_broadcast(
                            (128, 4, HEADS, HD)
                        )
                        if jp == 0:
                            nc.vector.scalar_tensor_tensor(
                                o4[:], vb, 1.0, wb, OP.mult, OP.mult
                            )
                        else:
                            t = avtp.tile(
                                [128, 4, HEADS, HD], F16, tag=f"avt{jp % 2}"
                            )
                            nc.gpsimd.tensor_mul(t[:], vb, wb)
                            nc.vector.tensor_add(o4[:], o4[:], t[:])

                    # --- transpose oacc -> oT[hd, rows] (fp16, 1 cyc) ----
                    oT = oTp.tile([128, KO, 4 * 128], F16, tag="oT")
                    for j in range(4):
                        for ko in range(KO):
                            pt = tpp.tile([128, 128], F16, tag="tp")
                            nc.tensor.transpose(
                                pt[:], oacc[:, j, ko * 128:(ko + 1) * 128], id_sb[:]
                            )
                            if (j * KO + ko) % 3 == 0:
                                nc.vector.tensor_copy(
                                    oT[:, ko, j * 128:(j + 1) * 128], pt[:]
                                )
                            else:
                                nc.scalar.copy(
                                    oT[:, ko, j * 128:(j + 1) * 128], pt[:]
                                )
                    oT_blocks.append(oT)

                # =============== out-projection ==========================
                ws_sb = wsp.tile([128, KO, EMBED], F16)
                for ko in range(KO):
                    nc.vector.tensor_scalar_mul(
                        ws_sb[:, ko, :], wo_sb[:, ko, :], r_sb[:, ko:ko + 1]
                    )

                for blk in range(NBLK):
                    oT = oT_blocks[blk]
                    for rc in range(4):
                        for half in range(2):
                            pf = mmp.tile([128, 384], F32, tag="mm")
                            for ko in range(KO):
                                nc.tensor.matmul(
                                    pf[:],
                                    lhsT=oT[:, ko, rc * 128:(rc + 1) * 128],
                                    rhs=ws_sb[:, ko, half * 384:(half + 1) * 384],
                                    start=(ko == 0),
                                    stop=(ko == KO - 1),
                                )
                            fin = finp.tile([128, 384], F32, tag="fin")
                            nc.scalar.copy(fin[:], pf[:])
                            rows = blk * 512 + rc * 128
                            nc.sync.dma_start(
                                y_d[rows:rows + 128, half * 384:(half + 1) * 384],
                                fin[:],
                            )

    nc.finalize()
    return nc


def _host_shard(x, Wqkv, Wout):
    """Build per-core input maps."""
    x = np.ascontiguousarray(np.asarray(x, dtype=np.float32))
    Wqkv = np.asarray(Wqkv, dtype=np.float32)
    Wout = np.asarray(Wout, dtype=np.float32)

    wq = np.ascontiguousarray(
        Wqkv.T.reshape(KO, 128, 3 * EMBED).transpose(1, 0, 2)
    )
    import concourse.mybir as _mybir
    F8NP = _mybir.dt.np(_mybir.dt.float8e5)
    wqk = wq[:, :, :QKW]
    wqh = np.ascontiguousarray(wqk.astype(np.float16))
    wql32 = wqk - wqh.astype(np.float32)
    # wq8[:, ko, 0, :] = wl*1024 (pairs with xh/1024); [:, ko, 1, :] = wh/32
    wq8 = np.empty((128, KO, 2, QKW), dtype=F8NP)
    wq8[:, :, 0, :] = (wql32 * 1024.0).astype(F8NP)
    wq8[:, :, 1, :] = (wqh.astype(np.float32) / 32.0).astype(F8NP)
    wv = wq[:, :, QKW:]
    wvh = np.ascontiguousarray(wv.astype(np.float16))
    wvl = np.ascontiguousarray((wv - wvh.astype(np.float32)).astype(np.float16))
    wo16 = np.ascontiguousarray(
        Wout.T.reshape(KO, 128, EMBED).transpose(1, 0, 2).astype(np.float16)
    )
    m2 = (np.arange(128) % 2 == 0).astype(np.float32).reshape(128, 1)
    m4 = (np.arange(128) % 4 == 0).astype(np.float32).reshape(128, 1)
    ident16 = np.eye(128, dtype=np.float16)

    in_maps = []
    for c in range(NCORES):
        bc, q = divmod(c, 4)
        xb = x[bc].reshape(4, 4, 4, 128, EMBED)  # [j, q, blk, g, e]
        mine = xb[:, q]                          # [j, blk, g, e]
        t = np.ascontiguousarray(mine.transpose(1, 0, 2, 3)).reshape(
            NCHUNK, 128, EMBED
        )
        xc = np.ascontiguousarray(
            t.reshape(NCHUNK, 128, KO, 128).transpose(0, 3, 2, 1)
        )
        xch = xc.astype(np.float16)
        xcl32 = xc - xch.astype(np.float32)
        x8 = np.empty((NCHUNK, 128, KO, 2, 128), dtype=F8NP)
        x8[:, :, :, 0, :] = (xch.astype(np.float32) / 1024.0).astype(F8NP)
        x8[:, :, :, 1, :] = (xcl32 * 32.0).astype(F8NP)
        xr = np.ascontiguousarray(t.reshape(NCHUNK, 128, KO, 128))
        xrh = xr.astype(np.float16)
        xrl = (xr - xrh.astype(np.float32)).astype(np.float16)
        in_maps.append(
            {
                "xch": xch, "x8": x8, "xrh": xrh, "xrl": xrl,
                "wqh": wqh, "wq8": wq8, "wvh": wvh, "wvl": wvl,
                "wo16": wo16, "m2": m2, "m4": m4, "ident16": ident16,
            }
        )
    return in_maps


def _host_assemble(results):
    y = np.empty((B, N, EMBED), dtype=np.float32)
    for c in range(NCORES):
        bc, q = divmod(c, 4)
        yc = np.asarray(results[c]["y"])  # [2048, 768], rows (blk, j, g)
        part = yc.reshape(4, 4, 128, EMBED).transpose(1, 0, 2, 3)  # [j, blk, g, e]
        y[bc].reshape(4, 4, 4, 128, EMBED)[:, q] = part
    return y


def kernel(x, Wqkv, Wout):
    from concourse.bass_utils import run_bass_kernel_spmd

    if "nc" not in _COMPILED:
        _COMPILED["nc"] = _build_program()
    nc = _COMPILED["nc"]

    in_maps = _host_shard(x, Wqkv, Wout)
    res = run_bass_kernel_spmd(nc, in_maps, core_ids=list(range(NCORES)))
    _COMPILED["last_result"] = res
    return _host_assemble(res.results)


if __name__ == "__main__":
    # smoke build
    nc = _build_program()
    print("built ok; instructions:", len(nc.inst_map))


# revision 62
# speedup vs baseline: 1.1138x; 1.0184x over previous
"""Dilated multi-head attention (nn_DilatedMHA) on 8 trn2 NeuronCores.

Math (reference restructured):
  qkv = x @ Wqkv.T                      [b, n, 3, h, d]   b=2, n=8192, h=12, d=64
  Position i attends within its mod-2048 class {p, p+2048, p+4096, p+6144}
  (p = i % 2048).  Per group p and head: r=1 full 4x4 softmax; r=2 (p even)
  2x2 among same-parity slots; r=4 (p%4==0) adds v.  out is normalized by
  its sum over the sequence per (b, h*d) channel, then projected by Wout.

Sharding: core c <- batch c//4, groups p in [(c%4)*512, (c%4)*512+512).

Key precision structure: the normalization denominator s nearly cancels
(min |s| ~ 0.018 vs ~0.5 summands), so anything that feeds s is amplified
~1000x into the output.  Instead of summing the (possibly noisy) attention
output, s is computed on an exact side channel:
    s[h,d] = sum_p sum_jp C[p,h,jp] * (x[p,jp,:] @ Wv[:, (h,d)])
           = gT.T @ Wv-diag-blocks,   gT = sum_chunks x_chunk.T @ C_chunk
where C = sum_j Wt are the attention-weight column sums (fp32).  C needs
~2^-15-accurate scores; Q/K are projected as xh@wh in fp16 (1 cyc/col)
plus BOTH hi/lo cross terms in one fp8e5m2 DoubleRow matmul (0.5 cyc/col,
two K-slots per PE cell): slot0 = (xh/1024)@(wl*1024), slot1 =
(xl*32)@(wh/32) - per-operand scales cancel within each slot and keep the
tiny residuals inside e5m2's normal range.  Measured on hardware this adds
only ~1e-3 of output error (the numpy e5m2 emulation predicted 1.3e-2 -
the real PE is kinder than the emulation).

With s exact, every other path only needs ~12-bit relative accuracy and runs
at the PE's full 1 cycle/column fp16 rate: the V projection is a single fp16
matmul, AV/oacc/oT/out-projection are fp16 (2x DVE throughput, 1 cyc/col
transposes), and Wout is fp16 with 1/s folded in per channel.

Structure: pass 0 = Q,K 3-term projections + per-pair scores (Pool muls /
DVE reduces chase each chunk's PSUM evacuation) + softmax/Wt/C + g; the
s side channel + AllGather (1.875x cheaper than AllReduce) then overlap
pass 1 = V + AV + transposes + out-projection.  Block 3's g-matmuls are
deferred into pass 1 so their ~10us wait on C3 does not overflow the PE's
4-deep wait queue at the pass boundary.  PSUM start/stop flags ride on
full-tile zero-init matmuls because the tile scheduler freely reorders
per-slice accumulating matmuls (a slice-level `start` can execute late and
wipe earlier contributions).

Measured: rel err 1.9e-3 on hardware (budget 2e-2); cost-model device
time 200801 ns vs 415908 ns baseline (2.07x).  PE busy 162us (75%):
QK 92us (fp16 + fp8-DR), V 31us, out-proj 31us, transposes 5us, g 1.4us
(the gT matmuls use the row-major xr tiles as the STATIONARY operand and
C as the 12-column moving operand, so they stream N=12 and land directly
in s-dot layout - no transposes).  fin bufs=4 matters: at 2 the
out-projection runs at 65% PE, gated by y-DMA latency.  Remaining idle:
~13us DMA head fill (4.7MB of QK weights), ~14us collective window,
drain.  Known rejected variants (each re-shuffles the tile scheduler
into a worse global order, +2..15us): DMA-XBAR transposes for oT, fp32
s-dot, AllReduce, sliced weight-DMA priority, split-half AV, oacc/vq
bufs=3, fold-on-Pool.  A 3-term fp16 QK fallback (rel err 9.0e-4,
308505 ns) is kernel_308.py.
"""

import sys

if "/opt/trn_rl_repo" not in sys.path:
    sys.path.insert(0, "/opt/trn_rl_repo")

import numpy as np

EMBED = 768
HEADS = 12
HD = 64
B = 2
N = 8192
NCORES = 8
GPC = 512           # groups per core
NBLK = 4            # blocks of 128 groups per core
NCHUNK = 16         # row chunks of 128 per core (blk, j)
KO = 6              # embed // 128
QKW = 2 * EMBED     # Q|K output columns

_COMPILED = {}


def _build_program():
    import concourse.mybir as mybir
    import concourse.tile as tile
    from concourse import bacc

    F32 = mybir.dt.float32
    F16 = mybir.dt.float16
    F8 = mybir.dt.float8e5
    DR = mybir.MatmulPerfMode.DoubleRow
    AX = mybir.AxisListType
    OP = mybir.AluOpType
    ACTF = mybir.ActivationFunctionType

    nc = bacc.Bacc("TRN2", target_bir_lowering=False, debug=False, num_devices=NCORES)

    # --- DRAM I/O ---------------------------------------------------------
    xch_d = nc.dram_tensor("xch", [NCHUNK, 128, KO, 128], F16, kind="ExternalInput")
    x8_d = nc.dram_tensor("x8", [NCHUNK, 128, KO, 2, 128], F8, kind="ExternalInput")
    xrh_d = nc.dram_tensor("xrh", [NCHUNK, 128, KO, 128], F16, kind="ExternalInput")
    xrl_d = nc.dram_tensor("xrl", [NCHUNK, 128, KO, 128], F16, kind="ExternalInput")
    wqh_d = nc.dram_tensor("wqh", [128, KO, QKW], F16, kind="ExternalInput")
    wq8_d = nc.dram_tensor("wq8", [128, KO, 2, QKW], F8, kind="ExternalInput")
    wvh_d = nc.dram_tensor("wvh", [128, KO, EMBED], F16, kind="ExternalInput")
    wvl_d = nc.dram_tensor("wvl", [128, KO, EMBED], F16, kind="ExternalInput")
    wo_d = nc.dram_tensor("wo16", [128, KO, EMBED], F16, kind="ExternalInput")
    m2_d = nc.dram_tensor("m2", [128, 1], F32, kind="ExternalInput")
    m4_d = nc.dram_tensor("m4", [128, 1], F32, kind="ExternalInput")
    id_d = nc.dram_tensor("ident16", [128, 128], F16, kind="ExternalInput")
    y_d = nc.dram_tensor("y", [4 * GPC, EMBED], F32, kind="ExternalOutput")

    with tile.TileContext(nc) as tc:
        with (
            tc.tile_pool(name="const", bufs=1) as constp,
            tc.tile_pool(name="oT", bufs=4) as oTp,
            tc.tile_pool(name="wt16", bufs=4) as wt16p,
            tc.tile_pool(name="xch", bufs=NCHUNK) as xchp,
            tc.tile_pool(name="mm", bufs=4, space="PSUM") as mmp,
            tc.tile_pool(name="gp", bufs=1, space="PSUM") as gpp,
            tc.tile_pool(name="dram", bufs=2, space="DRAM") as dramp,
            tc.tile_pool(name="xr", bufs=6) as xrp,
            tc.tile_pool(name="att", bufs=1) as attp,
        ):
            # --- long-lived SBUF -----------------------------------------
            wqh_sb = constp.tile([128, KO, QKW], F16)
            wq8_sb = constp.tile([128, KO, 2, QKW], F8)
            wvh_sb = constp.tile([128, KO, EMBED], F16)
            wvl_sb = constp.tile([128, KO, EMBED], F16)
            wo_sb = constp.tile([128, KO, EMBED], F16)
            m2_sb = constp.tile([128, 1], F32)
            nc.sync.dma_start(m2_sb[:], m2_d[:])
            m4_sb = constp.tile([128, 1], F32)
            nc.sync.dma_start(m4_sb[:], m4_d[:])
            id_sb = constp.tile([128, 128], F16)
            nc.sync.dma_start(id_sb[:], id_d[:])
            zero384 = constp.tile([128, 384], F16)
            nc.vector.memset(zero384[:], 0.0)

            # gT accumulator [e-slice, ko, h]: gT = sum_chunks x_chunk.T @ C,
            # with the row-major xr tiles as the stationary operand and C as
            # the 12-column moving operand (N=12 -> ~1.4us of PE total), and
            # the output directly in s-dot layout.  A full-tile zero-init
            # matmul carries the single `start` so every accumulating matmul
            # has a data dependency on it (the scheduler is otherwise free to
            # reorder, which would run the start late and wipe contributions).
            gps = gpp.tile([128, KO, HEADS], F32, tag="g")
            nc.tensor.matmul(
                gps[:].rearrange("p a b -> p (a b)"), lhsT=id_sb[:],
                rhs=zero384[:, 0:KO * HEADS], start=True, stop=False,
            )

            oT_blocks = []
            wt16_blocks = []
            xch_tiles = {}
            deferred_g = []

            def emit_g(blk, xhl, Ch, Cl):
                for j in range(4):
                    _, _, xrh_sb, xrl_sb = xhl[j]
                    last = blk == NBLK - 1 and j == 3
                    chj = Ch[:, :, j:j + 1].rearrange("p h a -> p (h a)")
                    clj = Cl[:, :, j:j + 1].rearrange("p h a -> p (h a)")
                    for ko in range(KO):
                        nc.tensor.matmul(
                            gps[:, ko, :], lhsT=xrh_sb[:, ko, :], rhs=chj,
                            start=False, stop=False,
                        )
                        nc.tensor.matmul(
                            gps[:, ko, :], lhsT=xrh_sb[:, ko, :], rhs=clj,
                            start=False, stop=False,
                        )
                        nc.tensor.matmul(
                            gps[:, ko, :], lhsT=xrl_sb[:, ko, :], rhs=chj,
                            start=False, stop=(last and ko == KO - 1),
                        )

            def load_xch(chunk):
                t = xchp.tile([128, KO, 128], F16, tag="xch")
                nc.sync.dma_start(t[:], xch_d[chunk])
                xch_tiles[chunk] = t
                return t

            # ============ Pass 0: Q,K + scores + C + g + s ===============
            with (
                tc.tile_pool(name="xcl", bufs=3) as xclp,
                tc.tile_pool(name="qkv", bufs=2) as qkvp,
                tc.tile_pool(name="prs", bufs=1) as prsp,
            ):
                def load_xr(chunk):
                    xrh_sb = xrp.tile([128, KO, 128], F16, tag="xrh")
                    nc.sync.dma_start(xrh_sb[:], xrh_d[chunk])
                    xrl_sb = xrp.tile([128, KO, 128], F16, tag="xrl")
                    nc.sync.dma_start(xrl_sb[:], xrl_d[chunk])
                    return xrh_sb, xrl_sb

                def load_qk_x(chunk):
                    xh_sb = load_xch(chunk)
                    x8_sb = xclp.tile([128, KO, 2, 128], F8, tag="x8")
                    nc.sync.dma_start(x8_sb[:], x8_d[chunk])
                    return xh_sb, x8_sb

                def load_chunk(chunk):
                    return load_qk_x(chunk) + load_xr(chunk)

                # DMA priority: chunk-0 QK x, QK weights, chunk 1, xr later.
                h0 = load_qk_x(0)
                for ko in range(KO):
                    nc.sync.dma_start(wqh_sb[:, ko, :], wqh_d[:, ko, :])
                    nc.sync.dma_start(wq8_sb[:, ko, :, :], wq8_d[:, ko, :, :])
                h1 = load_qk_x(1)
                pre = {0: h0 + load_xr(0), 1: h1 + load_xr(1)}

                for blk in range(NBLK):
                    if blk == 2:
                        # pass-1 weights, off the critical path at both ends
                        nc.sync.dma_start(wvh_sb[:], wvh_d[:])
                        nc.sync.dma_start(wvl_sb[:], wvl_d[:])
                        nc.sync.dma_start(wo_sb[:], wo_d[:])
                    Qb = qkvp.tile([128, 4, EMBED], F32, tag="qb")
                    Kb = qkvp.tile([128, 4, EMBED], F32, tag="kb")
                    Q4 = Qb[:].rearrange("p j (h d) -> p j h d", d=HD)
                    K4 = Kb[:].rearrange("p j (h d) -> p j h d", d=HD)
                    S = attp.tile([128, 4, HEADS, 4], F32, tag="S")
                    xhl = []
                    npair = 0
                    for j in range(4):
                        chunk = blk * 4 + j
                        if chunk in pre:
                            tiles = pre.pop(chunk)
                        else:
                            tiles = load_chunk(chunk)
                        if chunk + 2 < NCHUNK and (chunk + 2) not in pre:
                            pre[chunk + 2] = load_chunk(chunk + 2)
                        xh_sb, x8_sb, xrh_sb, xrl_sb = tiles
                        xhl.append(tiles)
                        # Q|K: fp16 main term + fp8e5 DoubleRow cross terms
                        # (slot0 = (xh/1024)@(wl*1024), slot1 = (xl*32)@(wh/32))
                        for n in range(4):
                            ps = mmp.tile([128, 384], F32, tag="mm")
                            sl = slice(n * 384, (n + 1) * 384)
                            for ko in range(KO):
                                nc.tensor.matmul(
                                    ps[:], lhsT=xh_sb[:, ko, :],
                                    rhs=wqh_sb[:, ko, sl],
                                    start=(ko == 0), stop=False,
                                )
                                nc.tensor.matmul(
                                    ps[:], lhsT=x8_sb[:, ko, :, :],
                                    rhs=wq8_sb[:, ko, :, sl],
                                    perf_mode=DR,
                                    start=False, stop=(ko == KO - 1),
                                )
                            dest = Qb if n < 2 else Kb
                            nc.scalar.copy(
                                dest[:, j, (n % 2) * 384:(n % 2 + 1) * 384], ps[:]
                            )

                        # per-pair scores chase the chunk evacuations
                        pairs = [(b_, j) for b_ in range(j + 1)]
                        pairs += [(j, c_) for c_ in range(j)]
                        for b_, c_ in pairs:
                            pr = prsp.tile(
                                [128, HEADS, HD], F32, tag=f"prs{npair % 2}"
                            )
                            nc.gpsimd.tensor_mul(pr[:], Q4[:, b_], K4[:, c_])
                            nc.vector.reduce_sum(
                                S[:, b_, :, c_], pr[:], axis=AX.X
                            )
                            npair += 1

                    E = attp.tile([128, 4, HEADS, 4], F32, tag="E")
                    nc.scalar.activation(E[:], S[:], ACTF.Exp, scale=0.125)

                    # --- attention weights Wt (fp32) ---------------------
                    Z1 = attp.tile([128, 4, HEADS], F32, tag="Z1")
                    nc.vector.reduce_sum(Z1[:], E[:], axis=AX.X)
                    R1 = attp.tile([128, 4, HEADS], F32, tag="R1")
                    nc.vector.reciprocal(R1[:], Z1[:])
                    Z2 = attp.tile([128, 4, HEADS, 2], F32, tag="Z2")
                    nc.vector.tensor_add(Z2[:], E[:, :, :, 0:2], E[:, :, :, 2:4])
                    R2 = attp.tile([128, 4, HEADS, 2], F32, tag="R2")
                    nc.vector.reciprocal(R2[:], Z2[:])

                    W1 = attp.tile([128, 4, HEADS, 4], F32, tag="W1")
                    nc.vector.tensor_mul(
                        W1[:], E[:], R1[:, :, :, None].to_broadcast((128, 4, HEADS, 4))
                    )
                    W2 = attp.tile([128, 4, HEADS, 4], F32, tag="W2")
                    nc.vector.memset(W2[:], 0.0)
                    for par in (0, 1):
                        nc.vector.tensor_mul(
                            W2[:, par::2, :, par::2],
                            E[:, par::2, :, par::2],
                            R2[:, par::2, :, par:par + 1].to_broadcast(
                                (128, 2, HEADS, 2)
                            ),
                        )
                    Wt = attp.tile([128, 4, HEADS, 4], F32, tag="Wt")
                    nc.vector.scalar_tensor_tensor(
                        Wt[:], W2[:], m2_sb[:, 0:1], W1[:], OP.mult, OP.add
                    )
                    for j in range(4):
                        nc.vector.tensor_scalar_add(
                            Wt[:, j, :, j:j + 1], Wt[:, j, :, j:j + 1], m4_sb[:, 0:1]
                        )
                    Wt16 = wt16p.tile([128, 4, HEADS, 4], F16, tag="wt16")
                    nc.scalar.copy(Wt16[:], Wt[:])
                    wt16_blocks.append(Wt16)

                    # --- C = column sums of Wt (exact, fp32) -------------
                    C = attp.tile([128, HEADS, 4], F32, tag="C")
                    nc.vector.reduce_sum(
                        C[:], Wt[:].rearrange("p j h k -> p h k j"), axis=AX.X
                    )
                    Ch = attp.tile([128, HEADS, 4], F16, tag="Ch")
                    nc.scalar.copy(Ch[:], C[:])
                    Cl = attp.tile([128, HEADS, 4], F16, tag="Cl")
                    nc.vector.tensor_sub(Cl[:], C[:], Ch[:])

                    # --- g[h, e] += C_chunk[p, h]^T @ x_chunk[p, e] -------
                    # 3-term fp16; xr tiles are row-major [p, (ko, e)].
                    # Block 3's g-matmuls wait ~10us on its C; deferring them
                    # into pass 1 keeps the PE wait queue from blocking the
                    # pass boundary.
                    if blk < NBLK - 1:
                        emit_g(blk, xhl, Ch, Cl)
                    else:
                        deferred_g.append((blk, xhl, Ch, Cl))

            # =============== exact s + AllGather (emitted mid pass 1) ====
            def emit_s_tail():
                for args in deferred_g:
                    emit_g(*args)
                g_sb = constp.tile([128, KO, HEADS], F32)
                nc.scalar.copy(g_sb[:], gps[:])
                gh = constp.tile([128, KO, HEADS], F16)
                nc.scalar.copy(gh[:], g_sb[:])
                gl = constp.tile([128, KO, HEADS], F16)
                nc.vector.tensor_sub(gl[:], g_sb[:], gh[:])

                # s matmuls accumulate into a gp-pool tile (tag reuse keeps
                # PSUM within 8 banks)
                stp = gpp.tile([128, KO, 2], F32, tag="gt")
                nc.tensor.matmul(
                    stp[:].rearrange("p a b -> p (a b)"), lhsT=id_sb[:],
                    rhs=zero384[:, 0:KO * 2], start=True, stop=False,
                )
                for t in range(KO):
                    sl = slice(t * 128, (t + 1) * 128)
                    hs = slice(2 * t, 2 * t + 2)
                    for ko in range(KO):
                        nc.tensor.matmul(
                            stp[:, t, :], lhsT=wvh_sb[:, ko, sl],
                            rhs=gh[:, ko, hs], start=False, stop=False,
                        )
                        nc.tensor.matmul(
                            stp[:, t, :], lhsT=wvh_sb[:, ko, sl],
                            rhs=gl[:, ko, hs], start=False, stop=False,
                        )
                        nc.tensor.matmul(
                            stp[:, t, :], lhsT=wvl_sb[:, ko, sl],
                            rhs=gh[:, ko, hs], start=False,
                            stop=(t == KO - 1 and ko == KO - 1),
                        )
                s_chan = constp.tile([128, KO], F32)
                for t in range(KO):
                    nc.vector.tensor_copy(s_chan[0:64, t:t + 1], stp[0:64, t, 0:1])
                    nc.vector.tensor_copy(
                        s_chan[64:128, t:t + 1], stp[64:128, t, 1:2]
                    )

                # AllGather (1.875x cheaper than AllReduce in latency) of the
                # four quarter-core partial sums, then add locally.
                cc_in = dramp.tile([128, KO], F32)
                cc_out = dramp.tile([4, 128, KO], F32)
                nc.gpsimd.dma_start(cc_in[:], s_chan[:])
                nc.gpsimd.collective_compute(
                    "AllGather",
                    OP.bypass,
                    replica_groups=[[0, 1, 2, 3], [4, 5, 6, 7]],
                    ins=[cc_in[:].opt()],
                    outs=[cc_out[:].opt()],
                )
                s_gath = constp.tile([128, 4, KO], F32)
                nc.gpsimd.dma_start(
                    s_gath[:], cc_out[:].rearrange("g p t -> p g t")
                )
                sa = constp.tile([128, KO], F32)
                nc.vector.tensor_add(sa[:], s_gath[:, 0, :], s_gath[:, 1, :])
                sb2 = constp.tile([128, KO], F32)
                nc.vector.tensor_add(sb2[:], s_gath[:, 2, :], s_gath[:, 3, :])
                s_tot = constp.tile([128, KO], F32)
                nc.vector.tensor_add(s_tot[:], sa[:], sb2[:])
                r_sb = constp.tile([128, KO], F32)
                nc.vector.reciprocal(r_sb[:], s_tot[:])
                return r_sb

            # ====== Pass 1: V + AV + transposes (collective hidden) ======
            with (
                tc.tile_pool(name="vq", bufs=2) as vqp,
                tc.tile_pool(name="avt", bufs=2) as avtp,
                tc.tile_pool(name="oacc", bufs=2) as oaccp,
                tc.tile_pool(name="ws", bufs=1) as wsp,
                tc.tile_pool(name="fin", bufs=6) as finp,
                tc.tile_pool(name="tp", bufs=2, space="PSUM") as tpp,
            ):
                r_sb = None
                for blk in range(NBLK):
                    if blk == 1:
                        r_sb = emit_s_tail()
                    V16 = vqp.tile([128, 4, EMBED], F16, tag="vb")
                    for j in range(4):
                        xh_sb = xch_tiles[blk * 4 + j]
                        for n in range(2):
                            ps = mmp.tile([128, 384], F32, tag="mm")
                            sl = slice(n * 384, (n + 1) * 384)
                            for ko in range(KO):
                                nc.tensor.matmul(
                                    ps[:], lhsT=xh_sb[:, ko, :],
                                    rhs=wvh_sb[:, ko, sl],
                                    start=(ko == 0), stop=(ko == KO - 1),
                                )
                            nc.scalar.copy(V16[:, j, n * 384:(n + 1) * 384], ps[:])

                    # --- AV (fp16): oacc[p, j] = sum_jp Wt16 * V16 -------
                    # muls on Pool (flat 0.833ns/elem, broadcast-immune),
                    # adds on DVE (fp16 packed 2x)
                    Wt16 = wt16_blocks[blk]
                    oacc = oaccp.tile([128, 4, EMBED], F16, tag="oacc")
                    o4 = oacc[:].rearrange("p j (h d) -> p j h d", d=HD)
                    for jp in range(4):
                        vb = (
                            V16[:, jp:jp + 1, :]
                            .rearrange("p a (h d) -> p a h d", d=HD)
                            .to_broadcast((128, 4, HEADS, HD))
                        )
                        wb = Wt16[:, :, :, jp:jp + 1].to_broadcast(
                            (128, 4, HEADS, HD)
                        )
                        if jp == 0:
                            nc.vector.scalar_tensor_tensor(
                                o4[:], vb, 1.0, wb, OP.mult, OP.mult
                            )
                        else:
                            t = avtp.tile(
                                [128, 4, HEADS, HD], F16, tag=f"avt{jp % 2}"
                            )
                            nc.gpsimd.tensor_mul(t[:], vb, wb)
                            nc.vector.tensor_add(o4[:], o4[:], t[:])

                    # --- transpose oacc -> oT[hd, rows] (fp16, 1 cyc) ----
                    oT = oTp.tile([128, KO, 4 * 128], F16, tag="oT")
                    for j in range(4):
                        for ko in range(KO):
                            pt = tpp.tile([128, 128], F16, tag="tp")
                            nc.tensor.transpose(
                                pt[:], oacc[:, j, ko * 128:(ko + 1) * 128], id_sb[:]
                            )
                            if (j * KO + ko) % 3 == 0:
                                nc.vector.tensor_copy(
                                    oT[:, ko, j * 128:(j + 1) * 128], pt[:]
                                )
                            else:
                                nc.scalar.copy(
                                    oT[:, ko, j * 128:(j + 1) * 128], pt[:]
                                )
                    oT_blocks.append(oT)

                # =============== out-projection ==========================
                ws_sb = wsp.tile([128, KO, EMBED], F16)
                for ko in range(KO):
                    nc.vector.tensor_scalar_mul(
                        ws_sb[:, ko, :], wo_sb[:, ko, :], r_sb[:, ko:ko + 1]
                    )

                for blk in range(NBLK):
                    oT = oT_blocks[blk]
                    for rc in range(4):
                        for half in range(2):
                            pf = mmp.tile([128, 384], F32, tag="mm")
                            for ko in range(KO):
                                nc.tensor.matmul(
                                    pf[:],
                                    lhsT=oT[:, ko, rc * 128:(rc + 1) * 128],
                                    rhs=ws_sb[:, ko, half * 384:(half + 1) * 384],
                                    start=(ko == 0),
                                    stop=(ko == KO - 1),
                                )
                            fin = finp.tile([128, 384], F32, tag="fin")
                            nc.scalar.copy(fin[:], pf[:])
                            rows = blk * 512 + rc * 128
                            nc.sync.dma_start(
                                y_d[rows:rows + 128, half * 384:(half + 1) * 384],
                                fin[:],
                            )

    nc.finalize()
    return nc


def _host_shard(x, Wqkv, Wout):
    """Build per-core input maps."""
    x = np.ascontiguousarray(np.asarray(x, dtype=np.float32))
    Wqkv = np.asarray(Wqkv, dtype=np.float32)
    Wout = np.asarray(Wout, dtype=np.float32)

    wq = np.ascontiguousarray(
        Wqkv.T.reshape(KO, 128, 3 * EMBED).transpose(1, 0, 2)
    )
    import concourse.mybir as _mybir
    F8NP = _mybir.dt.np(_mybir.dt.float8e5)
    wqk = wq[:, :, :QKW]
    wqh = np.ascontiguousarray(wqk.astype(np.float16))
    wql32 = wqk - wqh.astype(np.float32)
    # wq8[:, ko, 0, :] = wl*1024 (pairs with xh/1024); [:, ko, 1, :] = wh/32
    wq8 = np.empty((128, KO, 2, QKW), dtype=F8NP)
    wq8[:, :, 0, :] = (wql32 * 1024.0).astype(F8NP)
    wq8[:, :, 1, :] = (wqh.astype(np.float32) / 32.0).astype(F8NP)
    wv = wq[:, :, QKW:]
    wvh = np.ascontiguousarray(wv.astype(np.float16))
    wvl = np.ascontiguousarray((wv - wvh.astype(np.float32)).astype(np.float16))
    wo16 = np.ascontiguousarray(
        Wout.T.reshape(KO, 128, EMBED).transpose(1, 0, 2).astype(np.float16)
    )
    m2 = (np.arange(128) % 2 == 0).astype(np.float32).reshape(128, 1)
    m4 = (np.arange(128) % 4 == 0).astype(np.float32).reshape(128, 1)
    ident16 = np.eye(128, dtype=np.float16)

    in_maps = []
    for c in range(NCORES):
        bc, q = divmod(c, 4)
        xb = x[bc].reshape(4, 4, 4, 128, EMBED)  # [j, q, blk, g, e]
        mine = xb[:, q]                          # [j, blk, g, e]
        t = np.ascontiguousarray(mine.transpose(1, 0, 2, 3)).reshape(
            NCHUNK, 128, EMBED
        )
        xc = np.ascontiguousarray(
            t.reshape(NCHUNK, 128, KO, 128).transpose(0, 3, 2, 1)
        )
        xch = xc.astype(np.float16)
        xcl32 = xc - xch.astype(np.float32)
        x8 = np.empty((NCHUNK, 128, KO, 2, 128), dtype=F8NP)
        x8[:, :, :, 0, :] = (xch.astype(np.float32) / 1024.0).astype(F8NP)
        x8[:, :, :, 1, :] = (xcl32 * 32.0).astype(F8NP)
        xr = np.ascontiguousarray(t.reshape(NCHUNK, 128, KO, 128))
        xrh = xr.astype(np.float16)
        xrl = (xr - xrh.astype(np.float32)).astype(np.float16)
        in_maps.append(
            {
                "xch": xch, "x8": x8, "xrh": xrh, "xrl": xrl,
                "wqh": wqh, "wq8": wq8, "wvh": wvh, "wvl": wvl,
                "wo16": wo16, "m2": m2, "m4": m4, "ident16": ident16,
            }
        )
    return in_maps


def _host_assemble(results):
    y = np.empty((B, N, EMBED), dtype=np.float32)
    for c in range(NCORES):
        bc, q = divmod(c, 4)
        yc = np.asarray(results[c]["y"])  # [2048, 768], rows (blk, j, g)
        part = yc.reshape(4, 4, 128, EMBED).transpose(1, 0, 2, 3)  # [j, blk, g, e]
        y[bc].reshape(4, 4, 4, 128, EMBED)[:, q] = part
    return y


def kernel(x, Wqkv, Wout):
    from concourse.bass_utils import run_bass_kernel_spmd

    if "nc" not in _COMPILED:
        _COMPILED["nc"] = _build_program()
    nc = _COMPILED["nc"]

    in_maps = _host_shard(x, Wqkv, Wout)
    res = run_bass_kernel_spmd(nc, in_maps, core_ids=list(range(NCORES)))
    _COMPILED["last_result"] = res
    return _host_assemble(res.results)


if __name__ == "__main__":
    # smoke build
    nc = _build_program()
    print("built ok; instructions:", len(nc.inst_map))
